# revision 11
# baseline (speedup 1.0000x reference)
"""Deformable 2D feature aggregator — Trainium2 Bass kernel, 8-core SPMD. v2.

Problem: B=2, C=128, H=96, W=160, P=9 points, G=8 groups.
  value = conv1x1(feats); w = softmax over P of conv1x1(feats); offs = conv1x1(feats)
  pts = anchors + offs; out_proj(conv-weighted bilinear gather of value at pts).

Sharding: 8 cores = 2 batches x 4 query-slices, rotated pixel ring per core.

v3 design:
  - Vertical-pair DRAM scratch scr2[r] = [v(r), v(r+W)] (512B rows). A single
    dma_gather element of 1KB with elem_step=512B starting at row (y0*W+x0)
    covers the full 2x2 bilinear stencil -> ONE gather index per (query,
    point), quartering v1's Q7 descriptor-generation time.
  - dma_gather spread over all 4 SWDGE queues (each = its own Q7 core pair)
    so descriptor generation overlaps up to 4x.
  - Value projection bf16 channel-major (one stationary LDWEIGHTS, 512-wide
    moving), two strided-output xbar transpose-DMAs interleave the (0, +W)
    shifts into SBUF, one SWDGE DMA (0.34ns/desc) writes scr2. f32 projection
    for offsets/weights.
  - Combine: ACT pre-broadcasts kw over gc (kw16) so the DVE multiply runs in
    2x mode; contiguous pairwise tree-reduce in bf16.
"""
import sys

sys.path.insert(0, "/opt/trn_rl_repo")

import numpy as np
import ml_dtypes

import concourse.bass as bass
import concourse.bacc as bacc
import concourse.mybir as mybir
import concourse.tile as tile
from concourse.ap import AP

# problem constants (hardcoded per harness contract)
B, C, H, W = 2, 128, 96, 160
HW = H * W                     # 15360
P, G, GC = 9, 8, 16
NCORES = 8
QS = B * HW // NCORES          # 3840 queries per core
NT = QS // 128                 # 30 query tiles
TCH = 2                        # query tiles per gather chunk
NCH = NT // TCH                # 15 gather chunks
NJ = P                         # 9 stencil gathers per query
NIDX_CH = TCH * 128 * NJ       # 2304 gather indices per chunk
SHIFT = 1024.0                 # floor-bias (exact in f32 for our range)
NPXT = HW // 128               # 120 pixel tiles
NROW = HW + 128                # scr2 rows (one extra pixel tile backs idx+1)
FVLEN = 16384                  # fvalb padded length (wrap cols appended)
NVMM = FVLEN // 512            # 32 value matmuls
FPCH = 640                     # f32 proj feats chunk
ELEM = 4 * C                   # gather elem: 4 corners x C (bf16)

f32 = mybir.dt.float32
bf16 = mybir.dt.bfloat16
i16 = mybir.dt.int16
Alu = mybir.AluOpType
Act = mybir.ActivationFunctionType
Ax = mybir.AxisListType

_CACHE: dict = {}


def _build_nc(stage=None):
    import os
    stage = stage or os.environ.get("BASS_STAGE", "full")
    nc = bacc.Bacc(num_swdge_queues=4)

    fvalb = nc.dram_tensor("fvalb", [C, FVLEN], bf16, kind="ExternalInput")
    fproj = nc.dram_tensor("fproj", [C, QS], f32, kind="ExternalInput")
    anch = nc.dram_tensor("anch", [128, NT * 2], f32, kind="ExternalInput")
    vwTb = nc.dram_tensor("vwTb", [C, C], bf16, kind="ExternalInput")
    w90T = nc.dram_tensor("w90T", [C, 90], f32, kind="ExternalInput")
    owTb = nc.dram_tensor("owTb", [C, C], bf16, kind="ExternalInput")
    b90r = nc.dram_tensor("b90r", [128, 90], f32, kind="ExternalInput")
    bvr = nc.dram_tensor("bvr", [128, C], f32, kind="ExternalInput")
    outb = nc.dram_tensor("outb", [128, 1], f32, kind="ExternalInput")
    oneh = nc.dram_tensor("oneh", [128, 8 * 128], f32, kind="ExternalInput")
    ident = nc.dram_tensor("ident", [128, 128], bf16, kind="ExternalInput")
    rotoff = nc.dram_tensor("rotoff", [128, 1], f32, kind="ExternalInput")
    out_d = nc.dram_tensor("out", [C, QS], f32, kind="ExternalOutput")

    with tile.TileContext(nc) as tc, nc.allow_low_precision("bf16 combine by design"):
        with (
            tc.tile_pool(name="const", bufs=1) as cpool,
            tc.tile_pool(name="stage", bufs=1) as spool,
            tc.tile_pool(name="ps", bufs=1, space="PSUM") as pspool,
            tc.tile_pool(name="dram", bufs=1, space="DRAM") as dpool,
        ):
            # ---- persistent loads ----
            vwTb_s = cpool.tile([C, C], bf16)
            nc.sync.dma_start(vwTb_s[:], vwTb[:])
            w90T_s = cpool.tile([C, 90], f32)
            nc.sync.dma_start(w90T_s[:], w90T[:])
            owTb_s = cpool.tile([C, C], bf16)
            nc.sync.dma_start(owTb_s[:], owTb[:])
            b90_s = cpool.tile([128, 90], f32)
            nc.sync.dma_start(b90_s[:], b90r[:])
            bvr_s = cpool.tile([128, C], f32)
            nc.sync.dma_start(bvr_s[:], bvr[:])
            outb_s = cpool.tile([128, 1], f32)
            nc.sync.dma_start(outb_s[:], outb[:])
            oneh_s = cpool.tile([128, 8 * 128], f32)
            nc.sync.dma_start(oneh_s[:], oneh[:])
            ident_s = cpool.tile([128, 128], bf16)
            nc.sync.dma_start(ident_s[:], ident[:])
            anch_s = cpool.tile([128, NT * 2], f32)
            nc.sync.dma_start(anch_s[:], anch[:])
            rot_s = cpool.tile([128, 1], f32)
            nc.sync.dma_start(rot_s[:], rotoff[:])

            # vertical-pair scratch: row r = [v(r), v(r+W)] of the ring
            # (256 bf16 = 512B). A 1KB gather elem at step 512B from row r
            # yields corners [(r),(r+W),(r+1),(r+1+W)]. Rows beyond the valid
            # token range hold wrap junk (finite, never gathered).
            scr2 = dpool.tile([NROW, 2 * C], bf16)

            # whole-kernel staging tiles
            kw = spool.tile([128, NT * P * 4 * G], bf16)
            sumcoef = spool.tile([128, NT * G], f32)
            idx16 = spool.tile([128, NCH * (NIDX_CH // 16)], i16)

            # ---- phase A: value map (channel-major bf16) -> scr4 ----
            btmp_ctx = tc.tile_pool(name="btmp", bufs=1)
            bpool = btmp_ctx.__enter__()
            proj_s = bpool.tile([128, NT * 90], f32)
            with (
                tc.tile_pool(name="vmapp", bufs=1) as vmpool,
                tc.tile_pool(name="fb", bufs=2) as fbpool,
                tc.tile_pool(name="fp", bufs=2) as fppool,
            ):
                vmap = vmpool.tile([128, FVLEN], bf16)
                for m in range(NVMM):
                    if m % 4 == 0:
                        fbch = fbpool.tile([128, 2048], bf16, tag="fb", name=f"fb{m}")
                        nc.sync.dma_start(fbch[:], fvalb[:, m * 512 : m * 512 + 2048])
                    colb = (m % 4) * 512
                    vps = pspool.tile([128, 512], f32, tag="v", bufs=2, name=f"vps{m}")
                    nc.tensor.matmul(vps[:], vwTb_s[:], fbch[:, colb : colb + 512],
                                     start=True, stop=True)
                    if m % 2 == 0:
                        nc.scalar.copy(vmap[:, m * 512 : (m + 1) * 512], vps[:])
                    else:
                        nc.vector.tensor_copy(vmap[:, m * 512 : (m + 1) * 512], vps[:])

                # staging: stok[p, t, slot, c] = vmap[c, 128t + p + slot*W]
                # (xbar transposes with strided/interleaved outputs), then
                # SWDGE-generated DMAs write scr2 (rows interleave the 128
                # partitions, so descriptors are 512B; Q7 CounterMachine emits
                # them at ~0.34ns/desc vs HWDGE's ~14ns/desc). Quartered so
                # transposes overlap the matmul chain and the scr2 writes.
                NTT = NROW // 128
                QT = 31
                for qi, t0 in enumerate(range(0, NTT, QT)):
                    nt = min(QT, NTT - t0)
                    stokq = fbpool.tile([128, QT, 2, C], bf16, tag="stok", name=f"stok{qi}")
                    for slot, dlt in enumerate((0, W)):
                        o = AP(tensor=stokq.tensor,
                               offset=stokq[:, :, :, :].offset + slot * C,
                               ap=[[stokq[:, :, :, :].ap[0][0], 128], [2 * C, nt], [1, C]])
                        nc.sync.dma_start_transpose(
                            o, vmap[:, dlt + t0 * 128 : dlt + t0 * 128 + nt * 128])
                    i_ap = AP(tensor=stokq.tensor, offset=stokq[:, :, :, :].offset,
                              ap=[[stokq[:, :, :, :].ap[0][0], 128], [1, nt * 2 * C]])
                    o = AP(tensor=scr2.tensor, offset=scr2[:, :].offset + t0 * 128 * 2 * C,
                           ap=[[2 * C, 128], [128 * 2 * C, nt], [1, 2 * C]])
                    nc.gpsimd.dma_start(o, i_ap)

                # ---- f32 projection (weights/offsets) for this core's queries ----
                for t in range(NT):
                    if t % (FPCH // 128) == 0:
                        fpch = fppool.tile([128, FPCH], f32, tag="fp", name=f"fp{t}")
                        nc.sync.dma_start(fpch[:], fproj[:, t * 128 : t * 128 + FPCH])
                    col = (t % (FPCH // 128)) * 128
                    pps = pspool.tile([128, 90], f32, tag="p", bufs=2, name=f"pps{t}")
                    nc.tensor.matmul(pps[:], fpch[:, col : col + 128], w90T_s[:],
                                     start=True, stop=True)
                    nc.vector.tensor_tensor(
                        out=proj_s[:, t * 90 : (t + 1) * 90],
                        in0=pps[:],
                        in1=b90_s[:],
                        op=Alu.add,
                    )

            # ---- phase B: batched softmax / coords / weights (query-major) ----
            # proj_s free layout per tile t: [0,72) = wlog (pt*8+g), [72,90) = offs (pt*2+xy)
            wmax = bpool.tile([128, NT * G], f32)
            wl_gp = AP(tensor=proj_s.tensor, offset=proj_s[:, :].offset,
                       ap=[[proj_s[:, :].ap[0][0], 128], [90, NT], [1, G], [G, P]])
            nc.vector.tensor_reduce(out=wmax[:, :].rearrange("p (t g) -> p t g", g=G),
                                    in_=wl_gp, axis=Ax.X, op=Alu.max)
            smf = bpool.tile([128, NT * P * G], f32)
            wl_pg = AP(tensor=proj_s.tensor, offset=proj_s[:, :].offset,
                       ap=[[proj_s[:, :].ap[0][0], 128], [90, NT], [G, P], [1, G]])
            wmax_b = AP(tensor=wmax.tensor, offset=wmax[:, :].offset,
                        ap=[[wmax[:, :].ap[0][0], 128], [G, NT], [0, P], [1, G]])
            nc.vector.tensor_tensor(
                out=smf[:, :].rearrange("p (t q g) -> p t q g", q=P, g=G),
                in0=wl_pg, in1=wmax_b, op=Alu.subtract)
            nc.scalar.activation(smf[:], smf[:], Act.Exp)
            ssum = bpool.tile([128, NT * G], f32)
            sm_gp = AP(tensor=smf.tensor, offset=smf[:, :].offset,
                       ap=[[smf[:, :].ap[0][0], 128], [P * G, NT], [1, G], [G, P]])
            nc.vector.tensor_reduce(out=ssum[:, :].rearrange("p (t g) -> p t g", g=G),
                                    in_=sm_gp, axis=Ax.X, op=Alu.add)
            rcps = bpool.tile([128, NT * G], f32)
            nc.vector.reciprocal(rcps[:], ssum[:])
            wsm = bpool.tile([128, NT * P * G], bf16)
            rcp_b = AP(tensor=rcps.tensor, offset=rcps[:, :].offset,
                       ap=[[rcps[:, :].ap[0][0], 128], [G, NT], [0, P], [1, G]])
            nc.vector.tensor_tensor(
                out=wsm[:, :].rearrange("p (t q g) -> p t q g", q=P, g=G),
                in0=smf[:, :].rearrange("p (t q g) -> p t q g", q=P, g=G),
                in1=rcp_b, op=Alu.mult)

            # coords: px/py [128, NT*P] laid out (t, pt)
            NP_ = NT * P

            px = bpool.tile([128, NP_], f32)
            py = bpool.tile([128, NP_], f32)
            offs_x = AP(tensor=proj_s.tensor, offset=proj_s[:, :].offset + 72,
                        ap=[[proj_s[:, :].ap[0][0], 128], [90, NT], [2, P]])
            offs_y = AP(tensor=proj_s.tensor, offset=proj_s[:, :].offset + 73,
                        ap=[[proj_s[:, :].ap[0][0], 128], [90, NT], [2, P]])
            anx = AP(tensor=anch_s.tensor, offset=anch_s[:, :].offset,
                     ap=[[anch_s[:, :].ap[0][0], 128], [2, NT], [0, P]])
            any_ = AP(tensor=anch_s.tensor, offset=anch_s[:, :].offset + 1,
                      ap=[[anch_s[:, :].ap[0][0], 128], [2, NT], [0, P]])
            pxv = px[:, :].rearrange("p (t q) -> p t q", q=P)
            pyv = py[:, :].rearrange("p (t q) -> p t q", q=P)
            nc.vector.tensor_tensor(out=pxv, in0=offs_x, in1=anx, op=Alu.add)
            nc.vector.tensor_tensor(out=pyv, in0=offs_y, in1=any_, op=Alu.add)

            xp = bpool.tile([128, NP_], f32)
            yp = bpool.tile([128, NP_], f32)
            nc.scalar.activation(xp[:], px[:], Act.Copy, bias=SHIFT - 0.5, scale=float(W))
            nc.scalar.activation(yp[:], py[:], Act.Copy, bias=SHIFT - 0.5, scale=float(H))
            # floor via round(x-0.5): (x + (2^23-0.5)) - 2^23. At integer x the
            # half-even tie may floor one low with frac 1.0 — an equivalent
            # bilinear weighting, so interpolation is unchanged.
            MAGIC = float(1 << 23)
            xf = bpool.tile([128, NP_], f32)
            yf = bpool.tile([128, NP_], f32)
            nc.vector.tensor_scalar(out=xf[:], in0=xp[:], scalar1=MAGIC - 0.5,
                                    scalar2=MAGIC, op0=Alu.add, op1=Alu.subtract)
            nc.vector.tensor_scalar(out=yf[:], in0=yp[:], scalar1=MAGIC - 0.5,
                                    scalar2=MAGIC, op0=Alu.add, op1=Alu.subtract)
            wx = bpool.tile([128, NP_], f32)
            wy = bpool.tile([128, NP_], f32)
            nc.vector.tensor_tensor(out=wx[:], in0=xp[:], in1=xf[:], op=Alu.subtract)
            nc.vector.tensor_tensor(out=wy[:], in0=yp[:], in1=yf[:], op=Alu.subtract)

            # token coords, clamped: x in [0, W-2], y in [0, H-2]
            xg = bpool.tile([128, NP_], f32)
            nc.vector.tensor_scalar(out=xg[:], in0=xf[:], scalar1=SHIFT, scalar2=0.0,
                                    op0=Alu.subtract, op1=Alu.max)
            nc.vector.tensor_scalar(out=xg[:], in0=xg[:], scalar1=float(W - 2), scalar2=None, op0=Alu.min)
            yg = bpool.tile([128, NP_], f32)
            nc.vector.tensor_scalar(out=yg[:], in0=yf[:], scalar1=SHIFT, scalar2=0.0,
                                    op0=Alu.subtract, op1=Alu.max)
            nc.vector.tensor_scalar(out=yg[:], in0=yg[:], scalar1=float(H - 2), scalar2=None, op0=Alu.min)

            ux = bpool.tile([128, NP_], f32)
            uy = bpool.tile([128, NP_], f32)
            nc.scalar.activation(ux[:], wx[:], Act.Copy, bias=1.0, scale=-1.0)
            nc.scalar.activation(uy[:], wy[:], Act.Copy, bias=1.0, scale=-1.0)

            # validity masks with edge-clamp weight swap (x and y symmetric):
            # b0 = u*mA + w*mB ; b1 = w*mA + u*mC
            #   mA = [0 <= f <= L-2], mB = [f == -1], mC = [f == L-1]
            tA = bpool.tile([128, NP_], f32)
            tB = bpool.tile([128, NP_], f32)
            v1 = bpool.tile([128, NP_], f32)
            v2 = bpool.tile([128, NP_], f32)

            def edge_weights(bx, f, w_, u_, L):
                mA = bpool.tile([128, NP_], f32)
                nc.vector.tensor_scalar(out=tA[:], in0=f[:], scalar1=SHIFT, scalar2=None, op0=Alu.is_ge)
                nc.vector.tensor_scalar(out=tB[:], in0=f[:], scalar1=SHIFT + L - 2, scalar2=None, op0=Alu.is_le)
                nc.vector.tensor_tensor(out=mA[:], in0=tA[:], in1=tB[:], op=Alu.mult)
                mB = bpool.tile([128, NP_], f32)
                nc.vector.tensor_scalar(out=mB[:], in0=f[:], scalar1=SHIFT - 1.0, scalar2=None, op0=Alu.is_equal)
                mC = bpool.tile([128, NP_], f32)
                nc.vector.tensor_scalar(out=mC[:], in0=f[:], scalar1=SHIFT + L - 1, scalar2=None, op0=Alu.is_equal)
                b0 = AP(tensor=bx.tensor, offset=bx[:, :].offset,
                        ap=[[bx[:, :].ap[0][0], 128], [2, NP_]])
                b1 = AP(tensor=bx.tensor, offset=bx[:, :].offset + 1,
                        ap=[[bx[:, :].ap[0][0], 128], [2, NP_]])
                nc.vector.tensor_tensor(out=v1[:], in0=u_[:], in1=mA[:], op=Alu.mult)
                nc.vector.tensor_tensor(out=v2[:], in0=w_[:], in1=mB[:], op=Alu.mult)
                nc.vector.tensor_tensor(out=b0, in0=v1[:], in1=v2[:], op=Alu.add)
                nc.vector.tensor_tensor(out=v1[:], in0=w_[:], in1=mA[:], op=Alu.mult)
                nc.vector.tensor_tensor(out=v2[:], in0=u_[:], in1=mC[:], op=Alu.mult)
                nc.vector.tensor_tensor(out=b1, in0=v1[:], in1=v2[:], op=Alu.add)

            bx = bpool.tile([128, NP_ * 2], f32)   # (t, pt, side)
            by = bpool.tile([128, NP_ * 2], f32)   # (t, pt, row)
            edge_weights(bx, xf, wx, ux, W)
            edge_weights(by, yf, wy, uy, H)

            # gather supertoken indices (rotated): idx = (yg*W + xg - rotoff) mod HW
            idxf = bpool.tile([128, NP_], f32)    # (t, pt)
            r0t = bpool.tile([128, NP_], f32)
            nc.scalar.activation(r0t[:], yg[:], Act.Copy, bias=0.0, scale=float(W))
            nc.vector.tensor_tensor(out=idxf[:], in0=r0t[:], in1=xg[:], op=Alu.add)
            nc.vector.tensor_scalar(out=idxf[:], in0=idxf[:], scalar1=rot_s[:, 0:1],
                                    scalar2=None, op0=Alu.subtract)
            wrap = bpool.tile([128, NP_], f32)
            nc.vector.tensor_scalar(out=wrap[:], in0=idxf[:], scalar1=0.0,
                                    scalar2=float(HW), op0=Alu.is_lt, op1=Alu.mult)
            nc.vector.tensor_tensor(out=idxf[:], in0=idxf[:], in1=wrap[:], op=Alu.add)

            # cw[t, pt, side, row] = bx[t,pt,side] * by[t,pt,row]  (bf16)
            # (gather elem corner order is side-major: r, r+W, r+1, r+1+W)
            cw = bpool.tile([128, NT * P * 4], bf16)
            for side in range(2):
                bx_r = AP(tensor=bx.tensor, offset=bx[:, :].offset + side,
                          ap=[[bx[:, :].ap[0][0], 128], [2 * P, NT], [2, P], [0, 2]])
                by_v = AP(tensor=by.tensor, offset=by[:, :].offset,
                          ap=[[by[:, :].ap[0][0], 128], [2 * P, NT], [2, P], [1, 2]])
                cw_r = AP(tensor=cw.tensor, offset=cw[:, :].offset + 2 * side,
                          ap=[[cw[:, :].ap[0][0], 128], [4 * P, NT], [4, P], [1, 2]])
                nc.vector.tensor_tensor(out=cw_r, in0=bx_r, in1=by_v, op=Alu.mult)

            # kw[t, pt, rs, g] = cw[t, pt, rs] * wsm[t, pt, g]  (bf16)
            for rs in range(4):
                cw_rs = AP(tensor=cw.tensor, offset=cw[:, :].offset + rs,
                           ap=[[cw[:, :].ap[0][0], 128], [4 * P, NT], [4, P], [0, G]])
                w_v = AP(tensor=wsm.tensor, offset=wsm[:, :].offset,
                         ap=[[wsm[:, :].ap[0][0], 128], [P * G, NT], [G, P], [1, G]])
                kw_rs = AP(tensor=kw.tensor, offset=kw[:, :].offset + rs * G,
                           ap=[[kw[:, :].ap[0][0], 128], [4 * P * G, NT], [4 * G, P], [1, G]])
                nc.vector.tensor_tensor(out=kw_rs, in0=cw_rs, in1=w_v, op=Alu.mult)

            # sumcoef[t, g] = sum_pt wsm * (bx0+bx1)*(by0+by1)   (for value_b)
            bsx = bpool.tile([128, NP_], f32)
            bsy = bpool.tile([128, NP_], f32)
            bx0r = AP(tensor=bx.tensor, offset=bx[:, :].offset, ap=[[bx[:, :].ap[0][0], 128], [2, NP_]])
            bx1r = AP(tensor=bx.tensor, offset=bx[:, :].offset + 1, ap=[[bx[:, :].ap[0][0], 128], [2, NP_]])
            by0r = AP(tensor=by.tensor, offset=by[:, :].offset, ap=[[by[:, :].ap[0][0], 128], [2, NP_]])
            by1r = AP(tensor=by.tensor, offset=by[:, :].offset + 1, ap=[[by[:, :].ap[0][0], 128], [2, NP_]])
            nc.vector.tensor_tensor(out=bsx[:], in0=bx0r, in1=bx1r, op=Alu.add)
            nc.vector.tensor_tensor(out=bsy[:], in0=by0r, in1=by1r, op=Alu.add)
            bws = bpool.tile([128, NP_], bf16)
            nc.vector.tensor_tensor(out=bws[:], in0=bsx[:], in1=bsy[:], op=Alu.mult)
            wp = bpool.tile([128, NT * P * G], bf16)
            bws_b = AP(tensor=bws.tensor, offset=bws[:, :].offset,
                       ap=[[bws[:, :].ap[0][0], 128], [P, NT], [1, P], [0, G]])
            nc.vector.tensor_tensor(
                out=wp[:, :].rearrange("p (t q g) -> p t q g", q=P, g=G),
                in0=wsm[:, :].rearrange("p (t q g) -> p t q g", q=P, g=G),
                in1=bws_b, op=Alu.mult)
            wp_gp = AP(tensor=wp.tensor, offset=wp[:, :].offset,
                       ap=[[wp[:, :].ap[0][0], 128], [P * G, NT], [1, G], [G, P]])
            nc.vector.tensor_reduce(out=sumcoef[:, :].rearrange("p (t g) -> p t g", g=G),
                                    in_=wp_gp, axis=Ax.X, op=Alu.add)

            # ---- phase B2: idx16 build (PE permutation, 3 chunks per PSUM tile) ----
            # flat gather order l = slot*128 + q (slot = local (tt, pt)):
            # idx16[q%16, 8*j' + qh] = idxf[16qh + q%16, j'] globally.
            NGRP = 5
            for grp in range(NGRP):
                jlo = grp * 54           # 3 chunks x 18 slots
                i16ps = pspool.tile([128, 54 * 8], f32, tag="i16", bufs=2, name=f"i16ps{grp}")
                for qh in range(8):
                    outap = AP(tensor=i16ps.tensor, offset=i16ps[:, :].offset + qh,
                               ap=[[i16ps[:, :].ap[0][0], 128], [8, 54]])
                    nc.tensor.matmul(outap, oneh_s[:, qh * 128 : (qh + 1) * 128],
                                     idxf[:, jlo : jlo + 54],
                                     start=True, stop=True)
                nc.vector.tensor_copy(
                    idx16[:, jlo * 8 : (jlo + 54) * 8], i16ps[:])

            btmp_ctx.__exit__(None, None, None)

            # ---- phase C: gather + combine ----
            scr_src = AP(tensor=scr2.tensor, offset=scr2[:, :].offset,
                         ap=[[2 * C, NROW - 1], [1, ELEM]])

            with (
                tc.tile_pool(name="g", bufs=5) as gpool,
                tc.tile_pool(name="tree", bufs=2) as tpool,
                tc.tile_pool(name="aggp", bufs=2) as apool,
            ):
                n_ch = NCH if stage == "full" else int(stage)
                for ch in range(n_ch):
                    gt = gpool.tile([128, TCH * NJ, ELEM], bf16, tag="g", name=f"g{ch}")
                    nc.gpsimd.dma_gather(
                        gt[:, :, :], scr_src,
                        idx16[:, ch * (NIDX_CH // 16) : (ch + 1) * (NIDX_CH // 16)],
                        num_idxs=NIDX_CH, num_idxs_reg=NIDX_CH,
                        elem_size=ELEM, elem_step=2 * C, single_packet=False,
                        queue_num=ch % 4,
                    )

                    aggT2 = apool.tile([128, TCH * 128], bf16, tag="aggT", name=f"aggT{ch}")
                    for tt_ in range(TCH):
                        t = ch * TCH + tt_
                        gof = gt[:, :, :].offset + tt_ * NJ * ELEM
                        # kw16[q, (j, g, gc)] = kw[q, (j, g)] broadcast over gc
                        # (on ACT so the DVE multiply below runs in 2x mode)
                        kw16 = tpool.tile([128, 36 * C], bf16, tag="kw16", name=f"kw16_{t}")
                        kwb = AP(tensor=kw.tensor, offset=kw[:, :].offset + t * P * 4 * G,
                                 ap=[[kw[:, :].ap[0][0], 128], [G, 36], [1, G], [0, GC]])
                        k16v = AP(tensor=kw16.tensor, offset=kw16[:, :].offset,
                                  ap=[[kw16[:, :].ap[0][0], 128], [C, 36], [GC, G], [1, GC]])
                        nc.scalar.activation(k16v, kwb, Act.Copy)

                        # tp = gt * kw16  (all-contiguous bf16 -> DVE 2x)
                        tp = tpool.tile([128, 36 * C], bf16, tag="tp", name=f"tp{t}")
                        g_v = AP(tensor=gt.tensor, offset=gof,
                                 ap=[[gt[:, :, :].ap[0][0], 128], [1, 36 * C]])
                        nc.vector.tensor_tensor(out=tp[:], in0=g_v, in1=kw16[:], op=Alu.mult)

                        # pairwise tree reduce over the 36 corner blocks (2x mode)
                        def pair_add(dst, dof, src, sof, nblk):
                            i0 = AP(tensor=src.tensor, offset=src[:, :].offset + sof,
                                    ap=[[src[:, :].ap[0][0], 128], [2 * C, nblk], [1, C]])
                            i1 = AP(tensor=src.tensor, offset=src[:, :].offset + sof + C,
                                    ap=[[src[:, :].ap[0][0], 128], [2 * C, nblk], [1, C]])
                            o = AP(tensor=dst.tensor, offset=dst[:, :].offset + dof,
                                   ap=[[dst[:, :].ap[0][0], 128], [C, nblk], [1, C]])
                            nc.vector.tensor_tensor(out=o, in0=i0, in1=i1, op=Alu.add)

                        t1 = tpool.tile([128, 18 * C], bf16, tag="t1", name=f"t1_{t}")
                        pair_add(t1, 0, tp, 0, 18)
                        t2 = tp     # ping-pong: tp is dead after t1
                        pair_add(t2, 0, t1, 0, 9)
                        t3 = t1
                        pair_add(t3, 0, t2, 0, 4)
                        t4 = tpool.tile([128, 2 * C], bf16, tag="t4", name=f"t4_{t}")
                        pair_add(t4, 0, t3, 0, 2)
                        t5 = tpool.tile([128, C], bf16, tag="t5", name=f"t5_{t}")
                        pair_add(t5, 0, t4, 0, 1)

                        # ebias = value_b * sumcoef (per query, per group)
                        ebias = apool.tile([128, C], f32, tag="eb", name=f"eb{t}")
                        sc_v = AP(tensor=sumcoef.tensor, offset=sumcoef[:, :].offset + t * G,
                                  ap=[[sumcoef[:, :].ap[0][0], 128], [1, G], [0, GC]])
                        bv_v = bvr_s[:, :].rearrange("p (g c) -> p g c", g=G)
                        nc.vector.tensor_tensor(out=ebias[:, :].rearrange("p (g c) -> p g c", g=G),
                                                in0=sc_v, in1=bv_v, op=Alu.mult)
                        # agg = t5 + t2[block 8] + ebias
                        agg = apool.tile([128, C], bf16, tag="agg", name=f"agg{t}")
                        t2tail = AP(tensor=t2.tensor, offset=t2[:, :].offset + 8 * C,
                                    ap=[[t2[:, :].ap[0][0], 128], [1, C]])
                        nc.gpsimd.tensor_tensor(out=agg[:], in0=t5[:], in1=t2tail, op=Alu.add)
                        agg2 = apool.tile([128, C], bf16, tag="agg2", name=f"agg2{t}")
                        nc.gpsimd.tensor_tensor(out=agg2[:], in0=agg[:], in1=ebias[:], op=Alu.add)

                        # transpose -> [c, q] (bf16)
                        trps = pspool.tile([128, C], bf16, tag="tr", bufs=1, name=f"tr{t}")
                        nc.tensor.transpose(trps[:], agg2[:], ident_s[:])
                        nc.scalar.copy(aggT2[:, tt_ * 128 : (tt_ + 1) * 128], trps[:])

                    # batched out-projection for the chunk: [c_out, 256]
                    fops = pspool.tile([128, TCH * 128], f32, tag="fo", bufs=1, name=f"fo{ch}")
                    nc.tensor.matmul(fops[:], owTb_s[:], aggT2[:], start=True, stop=True)
                    fo_sb = apool.tile([128, TCH * 128], f32, tag="fosb", name=f"fosb{ch}")
                    nc.scalar.activation(fo_sb[:], fops[:], Act.Identity,
                                         bias=outb_s[:, 0:1], scale=1.0)
                    nc.sync.dma_start(out_d[:, ch * TCH * 128 : (ch + 1) * TCH * 128], fo_sb[:])

    nc.finalize()
    return nc


def _host_prep(inputs):
    """Prepare per-core input maps from full inputs."""
    feats = np.asarray(inputs["feats"], np.float32)          # [B, C, H, W]
    anchor = np.asarray(inputs["anchor_points"], np.float32)  # [B, HW, 2]
    value_w = np.asarray(inputs["value_w"], np.float32)
    value_b = np.asarray(inputs["value_b"], np.float32)
    weights_w = np.asarray(inputs["weights_w"], np.float32)
    weights_b = np.asarray(inputs["weights_b"], np.float32)
    offset_w = np.asarray(inputs["offset_w"], np.float32)
    offset_b = np.asarray(inputs["offset_b"], np.float32)
    out_w = np.asarray(inputs["out_w"], np.float32)
    out_b = np.asarray(inputs["out_b"], np.float32)

    w90 = np.concatenate([weights_w, offset_w], 0)            # [90, C]
    b90 = np.concatenate([weights_b, offset_b], 0)            # [90]
    shared = {
        "vwTb": np.ascontiguousarray(value_w.T).astype(ml_dtypes.bfloat16),
        "w90T": np.ascontiguousarray(w90.T),
        "owTb": np.ascontiguousarray(out_w.T).astype(ml_dtypes.bfloat16),
        "b90r": np.broadcast_to(b90, (128, 90)).copy(),
        "bvr": np.broadcast_to(value_b, (128, C)).copy(),
        "outb": out_b.reshape(128, 1).copy(),
        "ident": np.eye(128, dtype=ml_dtypes.bfloat16),
    }
    oneh = np.zeros((128, 8, 128), np.float32)
    for qh in range(8):
        for m in range(128):
            oneh[16 * qh + (m % 16), qh, m] = 1.0
    shared["oneh"] = oneh.reshape(128, 8 * 128)

    in_maps = []
    for core in range(NCORES):
        b_i, sl = core // 4, core % 4
        off = sl * QS
        fr = np.roll(feats[b_i].reshape(C, HW), -off, axis=1)
        fx = np.concatenate([fr, fr[:, : FVLEN - HW]], axis=1)
        an = anchor[b_i, off : off + QS].reshape(NT, 128, 2).transpose(1, 0, 2).reshape(128, NT * 2)
        m = dict(shared)
        m["fvalb"] = np.ascontiguousarray(fx).astype(ml_dtypes.bfloat16)
        m["fproj"] = np.ascontiguousarray(fr[:, :QS])
        m["anch"] = np.ascontiguousarray(an)
        m["rotoff"] = np.full((128, 1), float(off), np.float32)
        in_maps.append(m)
    return in_maps


def kernel(**inputs) -> np.ndarray:
    from concourse.bass_utils import run_bass_kernel_spmd

    if "nc" not in _CACHE:
        _CACHE["nc"] = _build_nc()
    nc = _CACHE["nc"]
    in_maps = _host_prep(inputs)
    res = run_bass_kernel_spmd(nc, in_maps, core_ids=list(range(NCORES)))
    out = np.zeros((B, C, HW), np.float32)
    for core in range(NCORES):
        b_i, sl = core // 4, core % 4
        out[b_i, :, sl * QS : (sl + 1) * QS] = res.results[core]["out"]
    return out.reshape(B, C, H, W)


# revision 12
# speedup vs baseline: 1.4281x; 1.4281x over previous
"""Deformable 2D feature aggregator — Trainium2 Bass kernel, 8-core SPMD. v2.

Problem: B=2, C=128, H=96, W=160, P=9 points, G=8 groups.
  value = conv1x1(feats); w = softmax over P of conv1x1(feats); offs = conv1x1(feats)
  pts = anchors + offs; out_proj(conv-weighted bilinear gather of value at pts).

Sharding: 8 cores = 2 batches x 4 query-slices, rotated pixel ring per core.

v3 design:
  - Vertical-pair DRAM scratch scr2[r] = [v(r), v(r+W)] (512B rows). A single
    dma_gather element of 1KB with elem_step=512B starting at row (y0*W+x0)
    covers the full 2x2 bilinear stencil -> ONE gather index per (query,
    point), quartering v1's Q7 descriptor-generation time.
  - dma_gather spread over all 4 SWDGE queues (each = its own Q7 core pair)
    so descriptor generation overlaps up to 4x.
  - Value projection bf16 channel-major (one stationary LDWEIGHTS, 512-wide
    moving), two strided-output xbar transpose-DMAs interleave the (0, +W)
    shifts into SBUF, one SWDGE DMA (0.34ns/desc) writes scr2. f32 projection
    for offsets/weights.
  - Combine: ACT pre-broadcasts kw over gc (kw16) so the DVE multiply runs in
    2x mode; contiguous pairwise tree-reduce in bf16.
"""
import sys

sys.path.insert(0, "/opt/trn_rl_repo")

import numpy as np
import ml_dtypes

import concourse.bass as bass
import concourse.bacc as bacc
import concourse.mybir as mybir
import concourse.tile as tile
from concourse.ap import AP

# problem constants (hardcoded per harness contract)
B, C, H, W = 2, 128, 96, 160
HW = H * W                     # 15360
P, G, GC = 9, 8, 16
NCORES = 8
QS = B * HW // NCORES          # 3840 queries per core
NT = QS // 128                 # 30 query tiles
TCH = 2                        # query tiles per gather chunk
NCH = NT // TCH                # 15 gather chunks
NJ = P                         # 9 stencil gathers per query
NIDX_CH = TCH * 128 * NJ       # 2304 gather indices per chunk
SHIFT = 1024.0                 # floor-bias (exact in f32 for our range)
NPXT = HW // 128               # 120 pixel tiles
NROW = HW + 128                # scr2 rows (one extra pixel tile backs idx+1)
FVLEN = 16384                  # fvalb padded length (wrap cols appended)
NVMM = FVLEN // 512            # 32 value matmuls
FPCH = 640                     # f32 proj feats chunk
ELEM = 4 * C                   # gather elem: 4 corners x C (bf16)

f32 = mybir.dt.float32
bf16 = mybir.dt.bfloat16
i16 = mybir.dt.int16
Alu = mybir.AluOpType
Act = mybir.ActivationFunctionType
Ax = mybir.AxisListType

_CACHE: dict = {}


def _build_nc(stage=None):
    import os
    stage = stage or os.environ.get("BASS_STAGE", "full")
    nc = bacc.Bacc(num_swdge_queues=4)

    fvalb = nc.dram_tensor("fvalb", [C, FVLEN], bf16, kind="ExternalInput")
    fproj = nc.dram_tensor("fproj", [C, QS], f32, kind="ExternalInput")
    anch = nc.dram_tensor("anch", [128, NT * 2], f32, kind="ExternalInput")
    vwTb = nc.dram_tensor("vwTb", [C, C], bf16, kind="ExternalInput")
    w90T = nc.dram_tensor("w90T", [C, 90], f32, kind="ExternalInput")
    owTb = nc.dram_tensor("owTb", [C, C], bf16, kind="ExternalInput")
    b90r = nc.dram_tensor("b90r", [128, 90], f32, kind="ExternalInput")
    bvr = nc.dram_tensor("bvr", [128, C], f32, kind="ExternalInput")
    outb = nc.dram_tensor("outb", [128, 1], f32, kind="ExternalInput")
    oneh = nc.dram_tensor("oneh", [128, 8 * 128], f32, kind="ExternalInput")
    ident = nc.dram_tensor("ident", [128, 128], bf16, kind="ExternalInput")
    rotoff = nc.dram_tensor("rotoff", [128, 1], f32, kind="ExternalInput")
    out_d = nc.dram_tensor("out", [C, QS], f32, kind="ExternalOutput")

    with tile.TileContext(nc) as tc, nc.allow_low_precision("bf16 combine by design"):
        with (
            tc.tile_pool(name="const", bufs=1) as cpool,
            tc.tile_pool(name="stage", bufs=1) as spool,
            tc.tile_pool(name="ps", bufs=1, space="PSUM") as pspool,
            tc.tile_pool(name="dram", bufs=1, space="DRAM") as dpool,
        ):
            # ---- persistent loads ----
            vwTb_s = cpool.tile([C, C], bf16)
            nc.sync.dma_start(vwTb_s[:], vwTb[:])
            w90T_s = cpool.tile([C, 90], f32)
            nc.sync.dma_start(w90T_s[:], w90T[:])
            owTb_s = cpool.tile([C, C], bf16)
            nc.sync.dma_start(owTb_s[:], owTb[:])
            b90_s = cpool.tile([128, 90], f32)
            nc.sync.dma_start(b90_s[:], b90r[:])
            bvr_s = cpool.tile([128, C], f32)
            nc.sync.dma_start(bvr_s[:], bvr[:])
            outb_s = cpool.tile([128, 1], f32)
            nc.sync.dma_start(outb_s[:], outb[:])
            oneh_s = cpool.tile([128, 8 * 128], f32)
            nc.sync.dma_start(oneh_s[:], oneh[:])
            ident_s = cpool.tile([128, 128], bf16)
            nc.sync.dma_start(ident_s[:], ident[:])
            anch_s = cpool.tile([128, NT * 2], f32)
            nc.sync.dma_start(anch_s[:], anch[:])
            rot_s = cpool.tile([128, 1], f32)
            nc.sync.dma_start(rot_s[:], rotoff[:])

            # vertical-pair scratch: row r = [v(r), v(r+W)] of the ring
            # (256 bf16 = 512B). A 1KB gather elem at step 512B from row r
            # yields corners [(r),(r+W),(r+1),(r+1+W)]. Rows beyond the valid
            # token range hold wrap junk (finite, never gathered).
            scr2 = dpool.tile([NROW, 2 * C], bf16)

            # whole-kernel staging tiles
            kw = spool.tile([128, NT * P * 4 * G], bf16)
            sumcoef = spool.tile([128, NT * G], f32)
            idx16 = spool.tile([128, NCH * (NIDX_CH // 16)], i16)

            # ---- phase A: value map (channel-major bf16) -> scr4 ----
            btmp_ctx = tc.tile_pool(name="btmp", bufs=1)
            bpool = btmp_ctx.__enter__()
            proj_s = bpool.tile([128, NT * 90], f32)
            with (
                tc.tile_pool(name="vmapp", bufs=1) as vmpool,
                tc.tile_pool(name="fb", bufs=2) as fbpool,
                tc.tile_pool(name="fp", bufs=2) as fppool,
            ):
                vmap = vmpool.tile([128, FVLEN], bf16)
                for m in range(NVMM):
                    if m % 4 == 0:
                        fbch = fbpool.tile([128, 2048], bf16, tag="fb", name=f"fb{m}")
                        nc.sync.dma_start(fbch[:], fvalb[:, m * 512 : m * 512 + 2048])
                    colb = (m % 4) * 512
                    vps = pspool.tile([128, 512], f32, tag="v", bufs=2, name=f"vps{m}")
                    nc.tensor.matmul(vps[:], vwTb_s[:], fbch[:, colb : colb + 512],
                                     start=True, stop=True)
                    if m % 2 == 0:
                        nc.scalar.copy(vmap[:, m * 512 : (m + 1) * 512], vps[:])
                    else:
                        nc.vector.tensor_copy(vmap[:, m * 512 : (m + 1) * 512], vps[:])

                # staging: stok[p, t, slot, c] = vmap[c, 128t + p + slot*W]
                # (xbar transposes with strided/interleaved outputs), then
                # SWDGE-generated DMAs write scr2 (rows interleave the 128
                # partitions, so descriptors are 512B; Q7 CounterMachine emits
                # them at ~0.34ns/desc vs HWDGE's ~14ns/desc). Quartered so
                # transposes overlap the matmul chain and the scr2 writes.
                NTT = NROW // 128
                QT = 31
                for qi, t0 in enumerate(range(0, NTT, QT)):
                    nt = min(QT, NTT - t0)
                    stokq = fbpool.tile([128, QT, 2, C], bf16, tag="stok", name=f"stok{qi}")
                    for slot, dlt in enumerate((0, W)):
                        o = AP(tensor=stokq.tensor,
                               offset=stokq[:, :, :, :].offset + slot * C,
                               ap=[[stokq[:, :, :, :].ap[0][0], 128], [2 * C, nt], [1, C]])
                        nc.sync.dma_start_transpose(
                            o, vmap[:, dlt + t0 * 128 : dlt + t0 * 128 + nt * 128])
                    i_ap = AP(tensor=stokq.tensor, offset=stokq[:, :, :, :].offset,
                              ap=[[stokq[:, :, :, :].ap[0][0], 128], [1, nt * 2 * C]])
                    o = AP(tensor=scr2.tensor, offset=scr2[:, :].offset + t0 * 128 * 2 * C,
                           ap=[[2 * C, 128], [128 * 2 * C, nt], [1, 2 * C]])
                    nc.gpsimd.dma_start(o, i_ap)

                # ---- f32 projection (weights/offsets) for this core's queries ----
                for t in range(NT):
                    if t % (FPCH // 128) == 0:
                        fpch = fppool.tile([128, FPCH], f32, tag="fp", name=f"fp{t}")
                        nc.sync.dma_start(fpch[:], fproj[:, t * 128 : t * 128 + FPCH])
                    col = (t % (FPCH // 128)) * 128
                    pps = pspool.tile([128, 90], f32, tag="p", bufs=2, name=f"pps{t}")
                    nc.tensor.matmul(pps[:], fpch[:, col : col + 128], w90T_s[:],
                                     start=True, stop=True)
                    nc.vector.tensor_tensor(
                        out=proj_s[:, t * 90 : (t + 1) * 90],
                        in0=pps[:],
                        in1=b90_s[:],
                        op=Alu.add,
                    )

            # ---- phase B: batched softmax / coords / weights (query-major) ----
            # proj_s free layout per tile t: [0,72) = wlog (pt*8+g), [72,90) = offs (pt*2+xy)
            wmax = bpool.tile([128, NT * G], f32)
            wl_gp = AP(tensor=proj_s.tensor, offset=proj_s[:, :].offset,
                       ap=[[proj_s[:, :].ap[0][0], 128], [90, NT], [1, G], [G, P]])
            nc.vector.tensor_reduce(out=wmax[:, :].rearrange("p (t g) -> p t g", g=G),
                                    in_=wl_gp, axis=Ax.X, op=Alu.max)
            smf = bpool.tile([128, NT * P * G], f32)
            wl_pg = AP(tensor=proj_s.tensor, offset=proj_s[:, :].offset,
                       ap=[[proj_s[:, :].ap[0][0], 128], [90, NT], [G, P], [1, G]])
            wmax_b = AP(tensor=wmax.tensor, offset=wmax[:, :].offset,
                        ap=[[wmax[:, :].ap[0][0], 128], [G, NT], [0, P], [1, G]])
            nc.vector.tensor_tensor(
                out=smf[:, :].rearrange("p (t q g) -> p t q g", q=P, g=G),
                in0=wl_pg, in1=wmax_b, op=Alu.subtract)
            nc.scalar.activation(smf[:], smf[:], Act.Exp)
            ssum = bpool.tile([128, NT * G], f32)
            sm_gp = AP(tensor=smf.tensor, offset=smf[:, :].offset,
                       ap=[[smf[:, :].ap[0][0], 128], [P * G, NT], [1, G], [G, P]])
            nc.vector.tensor_reduce(out=ssum[:, :].rearrange("p (t g) -> p t g", g=G),
                                    in_=sm_gp, axis=Ax.X, op=Alu.add)
            rcps = bpool.tile([128, NT * G], f32)
            nc.vector.reciprocal(rcps[:], ssum[:])
            wsm = bpool.tile([128, NT * P * G], bf16)
            rcp_b = AP(tensor=rcps.tensor, offset=rcps[:, :].offset,
                       ap=[[rcps[:, :].ap[0][0], 128], [G, NT], [0, P], [1, G]])
            nc.vector.tensor_tensor(
                out=wsm[:, :].rearrange("p (t q g) -> p t q g", q=P, g=G),
                in0=smf[:, :].rearrange("p (t q g) -> p t q g", q=P, g=G),
                in1=rcp_b, op=Alu.mult)

            # coords: px/py [128, NT*P] laid out (t, pt)
            NP_ = NT * P

            px = bpool.tile([128, NP_], f32)
            py = bpool.tile([128, NP_], f32)
            offs_x = AP(tensor=proj_s.tensor, offset=proj_s[:, :].offset + 72,
                        ap=[[proj_s[:, :].ap[0][0], 128], [90, NT], [2, P]])
            offs_y = AP(tensor=proj_s.tensor, offset=proj_s[:, :].offset + 73,
                        ap=[[proj_s[:, :].ap[0][0], 128], [90, NT], [2, P]])
            anx = AP(tensor=anch_s.tensor, offset=anch_s[:, :].offset,
                     ap=[[anch_s[:, :].ap[0][0], 128], [2, NT], [0, P]])
            any_ = AP(tensor=anch_s.tensor, offset=anch_s[:, :].offset + 1,
                      ap=[[anch_s[:, :].ap[0][0], 128], [2, NT], [0, P]])
            pxv = px[:, :].rearrange("p (t q) -> p t q", q=P)
            pyv = py[:, :].rearrange("p (t q) -> p t q", q=P)
            nc.vector.tensor_tensor(out=pxv, in0=offs_x, in1=anx, op=Alu.add)
            nc.vector.tensor_tensor(out=pyv, in0=offs_y, in1=any_, op=Alu.add)

            xp = bpool.tile([128, NP_], f32)
            yp = bpool.tile([128, NP_], f32)
            nc.scalar.activation(xp[:], px[:], Act.Copy, bias=SHIFT - 0.5, scale=float(W))
            nc.scalar.activation(yp[:], py[:], Act.Copy, bias=SHIFT - 0.5, scale=float(H))
            # floor via round(x-0.5): (x + (2^23-0.5)) - 2^23. At integer x the
            # half-even tie may floor one low with frac 1.0 — an equivalent
            # bilinear weighting, so interpolation is unchanged.
            MAGIC = float(1 << 23)
            xf = bpool.tile([128, NP_], f32)
            yf = bpool.tile([128, NP_], f32)
            nc.vector.tensor_scalar(out=xf[:], in0=xp[:], scalar1=MAGIC - 0.5,
                                    scalar2=MAGIC, op0=Alu.add, op1=Alu.subtract)
            nc.vector.tensor_scalar(out=yf[:], in0=yp[:], scalar1=MAGIC - 0.5,
                                    scalar2=MAGIC, op0=Alu.add, op1=Alu.subtract)
            wx = bpool.tile([128, NP_], f32)
            wy = bpool.tile([128, NP_], f32)
            nc.vector.tensor_tensor(out=wx[:], in0=xp[:], in1=xf[:], op=Alu.subtract)
            nc.vector.tensor_tensor(out=wy[:], in0=yp[:], in1=yf[:], op=Alu.subtract)

            # token coords, clamped: x in [0, W-2], y in [0, H-2]
            xg = bpool.tile([128, NP_], f32)
            nc.vector.tensor_scalar(out=xg[:], in0=xf[:], scalar1=SHIFT, scalar2=0.0,
                                    op0=Alu.subtract, op1=Alu.max)
            nc.vector.tensor_scalar(out=xg[:], in0=xg[:], scalar1=float(W - 2), scalar2=None, op0=Alu.min)
            yg = bpool.tile([128, NP_], f32)
            nc.vector.tensor_scalar(out=yg[:], in0=yf[:], scalar1=SHIFT, scalar2=0.0,
                                    op0=Alu.subtract, op1=Alu.max)
            nc.vector.tensor_scalar(out=yg[:], in0=yg[:], scalar1=float(H - 2), scalar2=None, op0=Alu.min)

            ux = bpool.tile([128, NP_], f32)
            uy = bpool.tile([128, NP_], f32)
            nc.scalar.activation(ux[:], wx[:], Act.Copy, bias=1.0, scale=-1.0)
            nc.scalar.activation(uy[:], wy[:], Act.Copy, bias=1.0, scale=-1.0)

            # validity masks with edge-clamp weight swap (x and y symmetric):
            # b0 = u*mA + w*mB ; b1 = w*mA + u*mC
            #   mA = [0 <= f <= L-2], mB = [f == -1], mC = [f == L-1]
            tA = bpool.tile([128, NP_], f32)
            tB = bpool.tile([128, NP_], f32)
            v1 = bpool.tile([128, NP_], f32)
            v2 = bpool.tile([128, NP_], f32)

            def edge_weights(bx, f, w_, u_, L):
                mA = bpool.tile([128, NP_], f32)
                nc.vector.tensor_scalar(out=tA[:], in0=f[:], scalar1=SHIFT, scalar2=None, op0=Alu.is_ge)
                nc.vector.tensor_scalar(out=tB[:], in0=f[:], scalar1=SHIFT + L - 2, scalar2=None, op0=Alu.is_le)
                nc.vector.tensor_tensor(out=mA[:], in0=tA[:], in1=tB[:], op=Alu.mult)
                mB = bpool.tile([128, NP_], f32)
                nc.vector.tensor_scalar(out=mB[:], in0=f[:], scalar1=SHIFT - 1.0, scalar2=None, op0=Alu.is_equal)
                mC = bpool.tile([128, NP_], f32)
                nc.vector.tensor_scalar(out=mC[:], in0=f[:], scalar1=SHIFT + L - 1, scalar2=None, op0=Alu.is_equal)
                b0 = AP(tensor=bx.tensor, offset=bx[:, :].offset,
                        ap=[[bx[:, :].ap[0][0], 128], [2, NP_]])
                b1 = AP(tensor=bx.tensor, offset=bx[:, :].offset + 1,
                        ap=[[bx[:, :].ap[0][0], 128], [2, NP_]])
                nc.vector.tensor_tensor(out=v1[:], in0=u_[:], in1=mA[:], op=Alu.mult)
                nc.vector.tensor_tensor(out=v2[:], in0=w_[:], in1=mB[:], op=Alu.mult)
                nc.vector.tensor_tensor(out=b0, in0=v1[:], in1=v2[:], op=Alu.add)
                nc.vector.tensor_tensor(out=v1[:], in0=w_[:], in1=mA[:], op=Alu.mult)
                nc.vector.tensor_tensor(out=v2[:], in0=u_[:], in1=mC[:], op=Alu.mult)
                nc.vector.tensor_tensor(out=b1, in0=v1[:], in1=v2[:], op=Alu.add)

            bx = bpool.tile([128, NP_ * 2], f32)   # (t, pt, side)
            by = bpool.tile([128, NP_ * 2], f32)   # (t, pt, row)
            edge_weights(bx, xf, wx, ux, W)
            edge_weights(by, yf, wy, uy, H)

            # gather supertoken indices (rotated): idx = (yg*W + xg - rotoff) mod HW
            idxf = bpool.tile([128, NP_], f32)    # (t, pt)
            r0t = bpool.tile([128, NP_], f32)
            nc.scalar.activation(r0t[:], yg[:], Act.Copy, bias=0.0, scale=float(W))
            nc.vector.tensor_tensor(out=idxf[:], in0=r0t[:], in1=xg[:], op=Alu.add)
            nc.vector.tensor_scalar(out=idxf[:], in0=idxf[:], scalar1=rot_s[:, 0:1],
                                    scalar2=None, op0=Alu.subtract)
            wrap = bpool.tile([128, NP_], f32)
            nc.vector.tensor_scalar(out=wrap[:], in0=idxf[:], scalar1=0.0,
                                    scalar2=float(HW), op0=Alu.is_lt, op1=Alu.mult)
            nc.vector.tensor_tensor(out=idxf[:], in0=idxf[:], in1=wrap[:], op=Alu.add)

            # cw[t, pt, side, row] = bx[t,pt,side] * by[t,pt,row]  (bf16)
            # (gather elem corner order is side-major: r, r+W, r+1, r+1+W)
            cw = bpool.tile([128, NT * P * 4], bf16)
            for side in range(2):
                bx_r = AP(tensor=bx.tensor, offset=bx[:, :].offset + side,
                          ap=[[bx[:, :].ap[0][0], 128], [2 * P, NT], [2, P], [0, 2]])
                by_v = AP(tensor=by.tensor, offset=by[:, :].offset,
                          ap=[[by[:, :].ap[0][0], 128], [2 * P, NT], [2, P], [1, 2]])
                cw_r = AP(tensor=cw.tensor, offset=cw[:, :].offset + 2 * side,
                          ap=[[cw[:, :].ap[0][0], 128], [4 * P, NT], [4, P], [1, 2]])
                nc.vector.tensor_tensor(out=cw_r, in0=bx_r, in1=by_v, op=Alu.mult)

            # kw[t, pt, rs, g] = cw[t, pt, rs] * wsm[t, pt, g]  (bf16)
            for rs in range(4):
                cw_rs = AP(tensor=cw.tensor, offset=cw[:, :].offset + rs,
                           ap=[[cw[:, :].ap[0][0], 128], [4 * P, NT], [4, P], [0, G]])
                w_v = AP(tensor=wsm.tensor, offset=wsm[:, :].offset,
                         ap=[[wsm[:, :].ap[0][0], 128], [P * G, NT], [G, P], [1, G]])
                kw_rs = AP(tensor=kw.tensor, offset=kw[:, :].offset + rs * G,
                           ap=[[kw[:, :].ap[0][0], 128], [4 * P * G, NT], [4 * G, P], [1, G]])
                nc.vector.tensor_tensor(out=kw_rs, in0=cw_rs, in1=w_v, op=Alu.mult)

            # sumcoef[t, g] = sum_pt wsm * (bx0+bx1)*(by0+by1)   (for value_b)
            bsx = bpool.tile([128, NP_], f32)
            bsy = bpool.tile([128, NP_], f32)
            bx0r = AP(tensor=bx.tensor, offset=bx[:, :].offset, ap=[[bx[:, :].ap[0][0], 128], [2, NP_]])
            bx1r = AP(tensor=bx.tensor, offset=bx[:, :].offset + 1, ap=[[bx[:, :].ap[0][0], 128], [2, NP_]])
            by0r = AP(tensor=by.tensor, offset=by[:, :].offset, ap=[[by[:, :].ap[0][0], 128], [2, NP_]])
            by1r = AP(tensor=by.tensor, offset=by[:, :].offset + 1, ap=[[by[:, :].ap[0][0], 128], [2, NP_]])
            nc.vector.tensor_tensor(out=bsx[:], in0=bx0r, in1=bx1r, op=Alu.add)
            nc.vector.tensor_tensor(out=bsy[:], in0=by0r, in1=by1r, op=Alu.add)
            bws = bpool.tile([128, NP_], bf16)
            nc.vector.tensor_tensor(out=bws[:], in0=bsx[:], in1=bsy[:], op=Alu.mult)
            wp = bpool.tile([128, NT * P * G], bf16)
            bws_b = AP(tensor=bws.tensor, offset=bws[:, :].offset,
                       ap=[[bws[:, :].ap[0][0], 128], [P, NT], [1, P], [0, G]])
            nc.vector.tensor_tensor(
                out=wp[:, :].rearrange("p (t q g) -> p t q g", q=P, g=G),
                in0=wsm[:, :].rearrange("p (t q g) -> p t q g", q=P, g=G),
                in1=bws_b, op=Alu.mult)
            wp_gp = AP(tensor=wp.tensor, offset=wp[:, :].offset,
                       ap=[[wp[:, :].ap[0][0], 128], [P * G, NT], [1, G], [G, P]])
            nc.vector.tensor_reduce(out=sumcoef[:, :].rearrange("p (t g) -> p t g", g=G),
                                    in_=wp_gp, axis=Ax.X, op=Alu.add)

            # ---- phase B2: idx16 build (PE permutation, 3 chunks per PSUM tile) ----
            # flat gather order l = slot*128 + q (slot = local (tt, pt)):
            # idx16[q%16, 8*j' + qh] = idxf[16qh + q%16, j'] globally.
            NGRP = 5
            for grp in range(NGRP):
                jlo = grp * 54           # 3 chunks x 18 slots
                i16ps = pspool.tile([128, 54 * 8], f32, tag="i16", bufs=2, name=f"i16ps{grp}")
                for qh in range(8):
                    outap = AP(tensor=i16ps.tensor, offset=i16ps[:, :].offset + qh,
                               ap=[[i16ps[:, :].ap[0][0], 128], [8, 54]])
                    nc.tensor.matmul(outap, oneh_s[:, qh * 128 : (qh + 1) * 128],
                                     idxf[:, jlo : jlo + 54],
                                     start=True, stop=True)
                nc.vector.tensor_copy(
                    idx16[:, jlo * 8 : (jlo + 54) * 8], i16ps[:])

            btmp_ctx.__exit__(None, None, None)

            # ---- phase C: gather + combine ----
            scr_src = AP(tensor=scr2.tensor, offset=scr2[:, :].offset,
                         ap=[[2 * C, NROW - 1], [1, ELEM]])

            with (
                tc.tile_pool(name="g", bufs=5) as gpool,
                tc.tile_pool(name="tree", bufs=2) as tpool,
                tc.tile_pool(name="aggp", bufs=2) as apool,
            ):
                n_ch = NCH if stage == "full" else int(stage)
                for ch in range(n_ch):
                    gt = gpool.tile([128, TCH * NJ, ELEM], bf16, tag="g", name=f"g{ch}")
                    nc.gpsimd.dma_gather(
                        gt[:, :, :], scr_src,
                        idx16[:, ch * (NIDX_CH // 16) : (ch + 1) * (NIDX_CH // 16)],
                        num_idxs=NIDX_CH, num_idxs_reg=NIDX_CH,
                        elem_size=ELEM, elem_step=2 * C, single_packet=False,
                        queue_num=ch % 4,
                    )

                    aggT2 = apool.tile([128, TCH * 128], bf16, tag="aggT", name=f"aggT{ch}")
                    for tt_ in range(TCH):
                        t = ch * TCH + tt_
                        gof = gt[:, :, :].offset + tt_ * NJ * ELEM
                        # kw16[q, (j, g, gc)] = kw[q, (j, g)] broadcast over gc
                        # (on ACT so the DVE multiply below runs in 2x mode)
                        kw16 = tpool.tile([128, 36 * C], bf16, tag="kw16", name=f"kw16_{t}")
                        kwb = AP(tensor=kw.tensor, offset=kw[:, :].offset + t * P * 4 * G,
                                 ap=[[kw[:, :].ap[0][0], 128], [G, 36], [1, G], [0, GC]])
                        k16v = AP(tensor=kw16.tensor, offset=kw16[:, :].offset,
                                  ap=[[kw16[:, :].ap[0][0], 128], [C, 36], [GC, G], [1, GC]])
                        nc.scalar.activation(k16v, kwb, Act.Copy)

                        # tp = gt * kw16  (all-contiguous bf16 -> DVE 2x)
                        tp = tpool.tile([128, 36 * C], bf16, tag="tp", name=f"tp{t}")
                        g_v = AP(tensor=gt.tensor, offset=gof,
                                 ap=[[gt[:, :, :].ap[0][0], 128], [1, 36 * C]])
                        nc.vector.tensor_tensor(out=tp[:], in0=g_v, in1=kw16[:], op=Alu.mult)

                        # pairwise tree reduce over the 36 corner blocks (2x mode)
                        def pair_add(dst, dof, src, sof, nblk):
                            i0 = AP(tensor=src.tensor, offset=src[:, :].offset + sof,
                                    ap=[[src[:, :].ap[0][0], 128], [2 * C, nblk], [1, C]])
                            i1 = AP(tensor=src.tensor, offset=src[:, :].offset + sof + C,
                                    ap=[[src[:, :].ap[0][0], 128], [2 * C, nblk], [1, C]])
                            o = AP(tensor=dst.tensor, offset=dst[:, :].offset + dof,
                                   ap=[[dst[:, :].ap[0][0], 128], [C, nblk], [1, C]])
                            nc.vector.tensor_tensor(out=o, in0=i0, in1=i1, op=Alu.add)

                        t1 = tpool.tile([128, 18 * C], bf16, tag="t1", name=f"t1_{t}")
                        pair_add(t1, 0, tp, 0, 18)
                        t2 = tp     # ping-pong: tp is dead after t1
                        pair_add(t2, 0, t1, 0, 9)
                        t3 = t1
                        pair_add(t3, 0, t2, 0, 4)
                        t4 = tpool.tile([128, 2 * C], bf16, tag="t4", name=f"t4_{t}")
                        pair_add(t4, 0, t3, 0, 2)
                        t5 = tpool.tile([128, C], bf16, tag="t5", name=f"t5_{t}")
                        pair_add(t5, 0, t4, 0, 1)

                        # ebias = value_b * sumcoef (per query, per group)
                        ebias = apool.tile([128, C], f32, tag="eb", name=f"eb{t}")
                        sc_v = AP(tensor=sumcoef.tensor, offset=sumcoef[:, :].offset + t * G,
                                  ap=[[sumcoef[:, :].ap[0][0], 128], [1, G], [0, GC]])
                        bv_v = bvr_s[:, :].rearrange("p (g c) -> p g c", g=G)
                        nc.vector.tensor_tensor(out=ebias[:, :].rearrange("p (g c) -> p g c", g=G),
                                                in0=sc_v, in1=bv_v, op=Alu.mult)
                        # agg = t5 + t2[block 8] + ebias
                        agg = apool.tile([128, C], bf16, tag="agg", name=f"agg{t}")
                        t2tail = AP(tensor=t2.tensor, offset=t2[:, :].offset + 8 * C,
                                    ap=[[t2[:, :].ap[0][0], 128], [1, C]])
                        nc.vector.tensor_tensor(out=agg[:], in0=t5[:], in1=t2tail, op=Alu.add)
                        agg2 = apool.tile([128, C], bf16, tag="agg2", name=f"agg2{t}")
                        nc.vector.tensor_tensor(out=agg2[:], in0=agg[:], in1=ebias[:], op=Alu.add)

                        # transpose -> [c, q] (bf16)
                        trps = pspool.tile([128, C], bf16, tag="tr", bufs=1, name=f"tr{t}")
                        nc.tensor.transpose(trps[:], agg2[:], ident_s[:])
                        nc.scalar.copy(aggT2[:, tt_ * 128 : (tt_ + 1) * 128], trps[:])

                    # batched out-projection for the chunk: [c_out, 256]
                    fops = pspool.tile([128, TCH * 128], f32, tag="fo", bufs=1, name=f"fo{ch}")
                    nc.tensor.matmul(fops[:], owTb_s[:], aggT2[:], start=True, stop=True)
                    fo_sb = apool.tile([128, TCH * 128], f32, tag="fosb", name=f"fosb{ch}")
                    nc.scalar.activation(fo_sb[:], fops[:], Act.Identity,
                                         bias=outb_s[:, 0:1], scale=1.0)
                    nc.sync.dma_start(out_d[:, ch * TCH * 128 : (ch + 1) * TCH * 128], fo_sb[:])

    nc.finalize()
    return nc


def _host_prep(inputs):
    """Prepare per-core input maps from full inputs."""
    feats = np.asarray(inputs["feats"], np.float32)          # [B, C, H, W]
    anchor = np.asarray(inputs["anchor_points"], np.float32)  # [B, HW, 2]
    value_w = np.asarray(inputs["value_w"], np.float32)
    value_b = np.asarray(inputs["value_b"], np.float32)
    weights_w = np.asarray(inputs["weights_w"], np.float32)
    weights_b = np.asarray(inputs["weights_b"], np.float32)
    offset_w = np.asarray(inputs["offset_w"], np.float32)
    offset_b = np.asarray(inputs["offset_b"], np.float32)
    out_w = np.asarray(inputs["out_w"], np.float32)
    out_b = np.asarray(inputs["out_b"], np.float32)

    w90 = np.concatenate([weights_w, offset_w], 0)            # [90, C]
    b90 = np.concatenate([weights_b, offset_b], 0)            # [90]
    shared = {
        "vwTb": np.ascontiguousarray(value_w.T).astype(ml_dtypes.bfloat16),
        "w90T": np.ascontiguousarray(w90.T),
        "owTb": np.ascontiguousarray(out_w.T).astype(ml_dtypes.bfloat16),
        "b90r": np.broadcast_to(b90, (128, 90)).copy(),
        "bvr": np.broadcast_to(value_b, (128, C)).copy(),
        "outb": out_b.reshape(128, 1).copy(),
        "ident": np.eye(128, dtype=ml_dtypes.bfloat16),
    }
    oneh = np.zeros((128, 8, 128), np.float32)
    for qh in range(8):
        for m in range(128):
            oneh[16 * qh + (m % 16), qh, m] = 1.0
    shared["oneh"] = oneh.reshape(128, 8 * 128)

    in_maps = []
    for core in range(NCORES):
        b_i, sl = core // 4, core % 4
        off = sl * QS
        fr = np.roll(feats[b_i].reshape(C, HW), -off, axis=1)
        fx = np.concatenate([fr, fr[:, : FVLEN - HW]], axis=1)
        an = anchor[b_i, off : off + QS].reshape(NT, 128, 2).transpose(1, 0, 2).reshape(128, NT * 2)
        m = dict(shared)
        m["fvalb"] = np.ascontiguousarray(fx).astype(ml_dtypes.bfloat16)
        m["fproj"] = np.ascontiguousarray(fr[:, :QS])
        m["anch"] = np.ascontiguousarray(an)
        m["rotoff"] = np.full((128, 1), float(off), np.float32)
        in_maps.append(m)
    return in_maps


def kernel(**inputs) -> np.ndarray:
    from concourse.bass_utils import run_bass_kernel_spmd

    if "nc" not in _CACHE:
        _CACHE["nc"] = _build_nc()
    nc = _CACHE["nc"]
    in_maps = _host_prep(inputs)
    res = run_bass_kernel_spmd(nc, in_maps, core_ids=list(range(NCORES)))
    out = np.zeros((B, C, HW), np.float32)
    for core in range(NCORES):
        b_i, sl = core // 4, core % 4
        out[b_i, :, sl * QS : (sl + 1) * QS] = res.results[core]["out"]
    return out.reshape(B, C, H, W)


# revision 13
# speedup vs baseline: 1.7327x; 1.2133x over previous
"""Deformable 2D feature aggregator — Trainium2 Bass kernel, 8-core SPMD. v2.

Problem: B=2, C=128, H=96, W=160, P=9 points, G=8 groups.
  value = conv1x1(feats); w = softmax over P of conv1x1(feats); offs = conv1x1(feats)
  pts = anchors + offs; out_proj(conv-weighted bilinear gather of value at pts).

Sharding: 8 cores = 2 batches x 4 query-slices, rotated pixel ring per core.

v3 design:
  - Vertical-pair DRAM scratch scr2[r] = [v(r), v(r+W)] (512B rows). A single
    dma_gather element of 1KB with elem_step=512B starting at row (y0*W+x0)
    covers the full 2x2 bilinear stencil -> ONE gather index per (query,
    point), quartering v1's Q7 descriptor-generation time.
  - dma_gather spread over all 4 SWDGE queues (each = its own Q7 core pair)
    so descriptor generation overlaps up to 4x.
  - Value projection bf16 channel-major (one stationary LDWEIGHTS, 512-wide
    moving), two strided-output xbar transpose-DMAs interleave the (0, +W)
    shifts into SBUF, one SWDGE DMA (0.34ns/desc) writes scr2. f32 projection
    for offsets/weights.
  - Combine: ACT pre-broadcasts kw over gc (kw16) so the DVE multiply runs in
    2x mode; contiguous pairwise tree-reduce in bf16.
"""
import sys

sys.path.insert(0, "/opt/trn_rl_repo")

import numpy as np
import ml_dtypes

import concourse.bass as bass
import concourse.bacc as bacc
import concourse.mybir as mybir
import concourse.tile as tile
from concourse.ap import AP

# problem constants (hardcoded per harness contract)
B, C, H, W = 2, 128, 96, 160
HW = H * W                     # 15360
P, G, GC = 9, 8, 16
NCORES = 8
QS = B * HW // NCORES          # 3840 queries per core
NT = QS // 128                 # 30 query tiles
TCH = 2                        # query tiles per gather chunk
NCH = NT // TCH                # 15 gather chunks
NJ = P                         # 9 stencil gathers per query
NIDX_CH = TCH * 128 * NJ       # 2304 gather indices per chunk
SHIFT = 1024.0                 # floor-bias (exact in f32 for our range)
NPXT = HW // 128               # 120 pixel tiles
NROW = HW + 128                # scr2 rows (one extra pixel tile backs idx+1)
FVLEN = 16384                  # fvalb padded length (wrap cols appended)
NVMM = FVLEN // 512            # 32 value matmuls
FPCH = 640                     # f32 proj feats chunk
ELEM = 4 * C                   # gather elem: 4 corners x C (bf16)

f32 = mybir.dt.float32
bf16 = mybir.dt.bfloat16
i16 = mybir.dt.int16
Alu = mybir.AluOpType
Act = mybir.ActivationFunctionType
Ax = mybir.AxisListType

_CACHE: dict = {}


def _build_nc(stage=None):
    import os
    stage = stage or os.environ.get("BASS_STAGE", "full")
    nc = bacc.Bacc(num_swdge_queues=4)

    fvalb = nc.dram_tensor("fvalb", [C, FVLEN], bf16, kind="ExternalInput")
    fproj = nc.dram_tensor("fproj", [C, QS], f32, kind="ExternalInput")
    anch = nc.dram_tensor("anch", [128, NT * 2], f32, kind="ExternalInput")
    vwTb = nc.dram_tensor("vwTb", [C, C], bf16, kind="ExternalInput")
    w90T = nc.dram_tensor("w90T", [C, 90], f32, kind="ExternalInput")
    owTb = nc.dram_tensor("owTb", [C, C], bf16, kind="ExternalInput")
    b90r = nc.dram_tensor("b90r", [128, 90], f32, kind="ExternalInput")
    bvr = nc.dram_tensor("bvr", [128, C], f32, kind="ExternalInput")
    outb = nc.dram_tensor("outb", [128, 1], f32, kind="ExternalInput")
    oneh = nc.dram_tensor("oneh", [128, 8 * 128], f32, kind="ExternalInput")
    ident = nc.dram_tensor("ident", [128, 128], bf16, kind="ExternalInput")
    rotoff = nc.dram_tensor("rotoff", [128, 1], f32, kind="ExternalInput")
    out_d = nc.dram_tensor("out", [C, QS], f32, kind="ExternalOutput")

    with tile.TileContext(nc) as tc, nc.allow_low_precision("bf16 combine by design"):
        with (
            tc.tile_pool(name="const", bufs=1) as cpool,
            tc.tile_pool(name="stage", bufs=1) as spool,
            tc.tile_pool(name="ps", bufs=1, space="PSUM") as pspool,
            tc.tile_pool(name="dram", bufs=1, space="DRAM") as dpool,
        ):
            # ---- persistent loads ----
            vwTb_s = cpool.tile([C, C], bf16)
            nc.sync.dma_start(vwTb_s[:], vwTb[:])
            w90T_s = cpool.tile([C, 90], f32)
            nc.sync.dma_start(w90T_s[:], w90T[:])
            owTb_s = cpool.tile([C, C], bf16)
            nc.sync.dma_start(owTb_s[:], owTb[:])
            b90_s = cpool.tile([128, 90], f32)
            nc.sync.dma_start(b90_s[:], b90r[:])
            bvr_s = cpool.tile([128, C], f32)
            nc.sync.dma_start(bvr_s[:], bvr[:])
            outb_s = cpool.tile([128, 1], f32)
            nc.sync.dma_start(outb_s[:], outb[:])
            oneh_s = cpool.tile([128, 8 * 128], f32)
            nc.sync.dma_start(oneh_s[:], oneh[:])
            ident_s = cpool.tile([128, 128], bf16)
            nc.sync.dma_start(ident_s[:], ident[:])
            anch_s = cpool.tile([128, NT * 2], f32)
            nc.sync.dma_start(anch_s[:], anch[:])
            rot_s = cpool.tile([128, 1], f32)
            nc.sync.dma_start(rot_s[:], rotoff[:])

            # vertical-pair scratch: row r = [v(r), v(r+W)] of the ring
            # (256 bf16 = 512B). A 1KB gather elem at step 512B from row r
            # yields corners [(r),(r+W),(r+1),(r+1+W)]. Rows beyond the valid
            # token range hold wrap junk (finite, never gathered).
            scr2 = dpool.tile([NROW, 2 * C], bf16)

            # whole-kernel staging tiles
            kw = spool.tile([128, NT * P * 4 * G], bf16)
            sumcoef = spool.tile([128, NT * G], f32)
            idx16 = spool.tile([128, NCH * (NIDX_CH // 16)], i16)

            # ---- phase A: value map (channel-major bf16) -> scr4 ----
            btmp_ctx = tc.tile_pool(name="btmp", bufs=1)
            bpool = btmp_ctx.__enter__()
            proj_s = bpool.tile([128, NT * 90], f32)
            with (
                tc.tile_pool(name="vmapp", bufs=1) as vmpool,
                tc.tile_pool(name="fb", bufs=2) as fbpool,
                tc.tile_pool(name="fp", bufs=2) as fppool,
            ):
                vmap = vmpool.tile([128, FVLEN], bf16)
                for m in range(NVMM):
                    if m % 4 == 0:
                        fbch = fbpool.tile([128, 2048], bf16, tag="fb", name=f"fb{m}")
                        nc.scalar.dma_start(fbch[:], fvalb[:, m * 512 : m * 512 + 2048])
                    colb = (m % 4) * 512
                    vps = pspool.tile([128, 512], f32, tag="v", bufs=2, name=f"vps{m}")
                    nc.tensor.matmul(vps[:], vwTb_s[:], fbch[:, colb : colb + 512],
                                     start=True, stop=True)
                    if m % 2 == 0:
                        nc.scalar.copy(vmap[:, m * 512 : (m + 1) * 512], vps[:])
                    else:
                        nc.vector.tensor_copy(vmap[:, m * 512 : (m + 1) * 512], vps[:])

                # staging: stok[p, t, slot, c] = vmap[c, 128t + p + slot*W]
                # (xbar transposes with strided/interleaved outputs), then
                # SWDGE-generated DMAs write scr2 (rows interleave the 128
                # partitions, so descriptors are 512B; Q7 CounterMachine emits
                # them at ~0.34ns/desc vs HWDGE's ~14ns/desc). Quartered so
                # transposes overlap the matmul chain and the scr2 writes.
                NTT = NROW // 128
                QT = 31
                for qi, t0 in enumerate(range(0, NTT, QT)):
                    nt = min(QT, NTT - t0)
                    stokq = fbpool.tile([128, QT, 2, C], bf16, tag="stok", name=f"stok{qi}")
                    for slot, dlt in enumerate((0, W)):
                        o = AP(tensor=stokq.tensor,
                               offset=stokq[:, :, :, :].offset + slot * C,
                               ap=[[stokq[:, :, :, :].ap[0][0], 128], [2 * C, nt], [1, C]])
                        nc.sync.dma_start_transpose(
                            o, vmap[:, dlt + t0 * 128 : dlt + t0 * 128 + nt * 128])
                    i_ap = AP(tensor=stokq.tensor, offset=stokq[:, :, :, :].offset,
                              ap=[[stokq[:, :, :, :].ap[0][0], 128], [1, nt * 2 * C]])
                    o = AP(tensor=scr2.tensor, offset=scr2[:, :].offset + t0 * 128 * 2 * C,
                           ap=[[2 * C, 128], [128 * 2 * C, nt], [1, 2 * C]])
                    nc.gpsimd.dma_start(o, i_ap)

                # ---- f32 projection (weights/offsets) for this core's queries ----
                for t in range(NT):
                    if t % (FPCH // 128) == 0:
                        fpch = fppool.tile([128, FPCH], f32, tag="fp", name=f"fp{t}")
                        nc.scalar.dma_start(fpch[:], fproj[:, t * 128 : t * 128 + FPCH])
                    col = (t % (FPCH // 128)) * 128
                    pps = pspool.tile([128, 90], f32, tag="p", bufs=2, name=f"pps{t}")
                    nc.tensor.matmul(pps[:], fpch[:, col : col + 128], w90T_s[:],
                                     start=True, stop=True)
                    nc.vector.tensor_tensor(
                        out=proj_s[:, t * 90 : (t + 1) * 90],
                        in0=pps[:],
                        in1=b90_s[:],
                        op=Alu.add,
                    )

            # ---- phase B: batched softmax / coords / weights (query-major) ----
            # proj_s free layout per tile t: [0,72) = wlog (pt*8+g), [72,90) = offs (pt*2+xy)
            wmax = bpool.tile([128, NT * G], f32)
            wl_gp = AP(tensor=proj_s.tensor, offset=proj_s[:, :].offset,
                       ap=[[proj_s[:, :].ap[0][0], 128], [90, NT], [1, G], [G, P]])
            nc.vector.tensor_reduce(out=wmax[:, :].rearrange("p (t g) -> p t g", g=G),
                                    in_=wl_gp, axis=Ax.X, op=Alu.max)
            smf = bpool.tile([128, NT * P * G], f32)
            wl_pg = AP(tensor=proj_s.tensor, offset=proj_s[:, :].offset,
                       ap=[[proj_s[:, :].ap[0][0], 128], [90, NT], [G, P], [1, G]])
            wmax_b = AP(tensor=wmax.tensor, offset=wmax[:, :].offset,
                        ap=[[wmax[:, :].ap[0][0], 128], [G, NT], [0, P], [1, G]])
            nc.vector.tensor_tensor(
                out=smf[:, :].rearrange("p (t q g) -> p t q g", q=P, g=G),
                in0=wl_pg, in1=wmax_b, op=Alu.subtract)
            nc.scalar.activation(smf[:], smf[:], Act.Exp)
            ssum = bpool.tile([128, NT * G], f32)
            sm_gp = AP(tensor=smf.tensor, offset=smf[:, :].offset,
                       ap=[[smf[:, :].ap[0][0], 128], [P * G, NT], [1, G], [G, P]])
            nc.vector.tensor_reduce(out=ssum[:, :].rearrange("p (t g) -> p t g", g=G),
                                    in_=sm_gp, axis=Ax.X, op=Alu.add)
            rcps = bpool.tile([128, NT * G], f32)
            nc.vector.reciprocal(rcps[:], ssum[:])
            wsm = bpool.tile([128, NT * P * G], bf16)
            rcp_b = AP(tensor=rcps.tensor, offset=rcps[:, :].offset,
                       ap=[[rcps[:, :].ap[0][0], 128], [G, NT], [0, P], [1, G]])
            nc.vector.tensor_tensor(
                out=wsm[:, :].rearrange("p (t q g) -> p t q g", q=P, g=G),
                in0=smf[:, :].rearrange("p (t q g) -> p t q g", q=P, g=G),
                in1=rcp_b, op=Alu.mult)

            # coords: px/py [128, NT*P] laid out (t, pt)
            NP_ = NT * P

            px = bpool.tile([128, NP_], f32)
            py = bpool.tile([128, NP_], f32)
            offs_x = AP(tensor=proj_s.tensor, offset=proj_s[:, :].offset + 72,
                        ap=[[proj_s[:, :].ap[0][0], 128], [90, NT], [2, P]])
            offs_y = AP(tensor=proj_s.tensor, offset=proj_s[:, :].offset + 73,
                        ap=[[proj_s[:, :].ap[0][0], 128], [90, NT], [2, P]])
            anx = AP(tensor=anch_s.tensor, offset=anch_s[:, :].offset,
                     ap=[[anch_s[:, :].ap[0][0], 128], [2, NT], [0, P]])
            any_ = AP(tensor=anch_s.tensor, offset=anch_s[:, :].offset + 1,
                      ap=[[anch_s[:, :].ap[0][0], 128], [2, NT], [0, P]])
            pxv = px[:, :].rearrange("p (t q) -> p t q", q=P)
            pyv = py[:, :].rearrange("p (t q) -> p t q", q=P)
            nc.vector.tensor_tensor(out=pxv, in0=offs_x, in1=anx, op=Alu.add)
            nc.vector.tensor_tensor(out=pyv, in0=offs_y, in1=any_, op=Alu.add)

            xp = bpool.tile([128, NP_], f32)
            yp = bpool.tile([128, NP_], f32)
            nc.scalar.activation(xp[:], px[:], Act.Copy, bias=SHIFT - 0.5, scale=float(W))
            nc.scalar.activation(yp[:], py[:], Act.Copy, bias=SHIFT - 0.5, scale=float(H))
            # floor via round(x-0.5): (x + (2^23-0.5)) - 2^23. At integer x the
            # half-even tie may floor one low with frac 1.0 — an equivalent
            # bilinear weighting, so interpolation is unchanged.
            MAGIC = float(1 << 23)
            xf = bpool.tile([128, NP_], f32)
            yf = bpool.tile([128, NP_], f32)
            nc.vector.tensor_scalar(out=xf[:], in0=xp[:], scalar1=MAGIC - 0.5,
                                    scalar2=MAGIC, op0=Alu.add, op1=Alu.subtract)
            nc.vector.tensor_scalar(out=yf[:], in0=yp[:], scalar1=MAGIC - 0.5,
                                    scalar2=MAGIC, op0=Alu.add, op1=Alu.subtract)
            wx = bpool.tile([128, NP_], f32)
            wy = bpool.tile([128, NP_], f32)
            nc.vector.tensor_tensor(out=wx[:], in0=xp[:], in1=xf[:], op=Alu.subtract)
            nc.vector.tensor_tensor(out=wy[:], in0=yp[:], in1=yf[:], op=Alu.subtract)

            # token coords, clamped: x in [0, W-2], y in [0, H-2]
            xg = bpool.tile([128, NP_], f32)
            nc.vector.tensor_scalar(out=xg[:], in0=xf[:], scalar1=SHIFT, scalar2=0.0,
                                    op0=Alu.subtract, op1=Alu.max)
            nc.vector.tensor_scalar(out=xg[:], in0=xg[:], scalar1=float(W - 2), scalar2=None, op0=Alu.min)
            yg = bpool.tile([128, NP_], f32)
            nc.vector.tensor_scalar(out=yg[:], in0=yf[:], scalar1=SHIFT, scalar2=0.0,
                                    op0=Alu.subtract, op1=Alu.max)
            nc.vector.tensor_scalar(out=yg[:], in0=yg[:], scalar1=float(H - 2), scalar2=None, op0=Alu.min)

            ux = bpool.tile([128, NP_], f32)
            uy = bpool.tile([128, NP_], f32)
            nc.scalar.activation(ux[:], wx[:], Act.Copy, bias=1.0, scale=-1.0)
            nc.scalar.activation(uy[:], wy[:], Act.Copy, bias=1.0, scale=-1.0)

            # validity masks with edge-clamp weight swap (x and y symmetric):
            # b0 = u*mA + w*mB ; b1 = w*mA + u*mC
            #   mA = [0 <= f <= L-2], mB = [f == -1], mC = [f == L-1]
            tA = bpool.tile([128, NP_], f32)
            tB = bpool.tile([128, NP_], f32)
            v1 = bpool.tile([128, NP_], f32)
            v2 = bpool.tile([128, NP_], f32)

            def edge_weights(bx, f, w_, u_, L):
                mA = bpool.tile([128, NP_], f32)
                nc.vector.tensor_scalar(out=tA[:], in0=f[:], scalar1=SHIFT, scalar2=None, op0=Alu.is_ge)
                nc.vector.tensor_scalar(out=tB[:], in0=f[:], scalar1=SHIFT + L - 2, scalar2=None, op0=Alu.is_le)
                nc.vector.tensor_tensor(out=mA[:], in0=tA[:], in1=tB[:], op=Alu.mult)
                mB = bpool.tile([128, NP_], f32)
                nc.vector.tensor_scalar(out=mB[:], in0=f[:], scalar1=SHIFT - 1.0, scalar2=None, op0=Alu.is_equal)
                mC = bpool.tile([128, NP_], f32)
                nc.vector.tensor_scalar(out=mC[:], in0=f[:], scalar1=SHIFT + L - 1, scalar2=None, op0=Alu.is_equal)
                b0 = AP(tensor=bx.tensor, offset=bx[:, :].offset,
                        ap=[[bx[:, :].ap[0][0], 128], [2, NP_]])
                b1 = AP(tensor=bx.tensor, offset=bx[:, :].offset + 1,
                        ap=[[bx[:, :].ap[0][0], 128], [2, NP_]])
                nc.vector.tensor_tensor(out=v1[:], in0=u_[:], in1=mA[:], op=Alu.mult)
                nc.vector.tensor_tensor(out=v2[:], in0=w_[:], in1=mB[:], op=Alu.mult)
                nc.vector.tensor_tensor(out=b0, in0=v1[:], in1=v2[:], op=Alu.add)
                nc.vector.tensor_tensor(out=v1[:], in0=w_[:], in1=mA[:], op=Alu.mult)
                nc.vector.tensor_tensor(out=v2[:], in0=u_[:], in1=mC[:], op=Alu.mult)
                nc.vector.tensor_tensor(out=b1, in0=v1[:], in1=v2[:], op=Alu.add)

            bx = bpool.tile([128, NP_ * 2], f32)   # (t, pt, side)
            by = bpool.tile([128, NP_ * 2], f32)   # (t, pt, row)
            edge_weights(bx, xf, wx, ux, W)
            edge_weights(by, yf, wy, uy, H)

            # gather supertoken indices (rotated): idx = (yg*W + xg - rotoff) mod HW
            idxf = bpool.tile([128, NP_], f32)    # (t, pt)
            r0t = bpool.tile([128, NP_], f32)
            nc.scalar.activation(r0t[:], yg[:], Act.Copy, bias=0.0, scale=float(W))
            nc.vector.tensor_tensor(out=idxf[:], in0=r0t[:], in1=xg[:], op=Alu.add)
            nc.vector.tensor_scalar(out=idxf[:], in0=idxf[:], scalar1=rot_s[:, 0:1],
                                    scalar2=None, op0=Alu.subtract)
            wrap = bpool.tile([128, NP_], f32)
            nc.vector.tensor_scalar(out=wrap[:], in0=idxf[:], scalar1=0.0,
                                    scalar2=float(HW), op0=Alu.is_lt, op1=Alu.mult)
            nc.vector.tensor_tensor(out=idxf[:], in0=idxf[:], in1=wrap[:], op=Alu.add)

            # cw[t, pt, side, row] = bx[t,pt,side] * by[t,pt,row]  (bf16)
            # (gather elem corner order is side-major: r, r+W, r+1, r+1+W)
            cw = bpool.tile([128, NT * P * 4], bf16)
            for side in range(2):
                bx_r = AP(tensor=bx.tensor, offset=bx[:, :].offset + side,
                          ap=[[bx[:, :].ap[0][0], 128], [2 * P, NT], [2, P], [0, 2]])
                by_v = AP(tensor=by.tensor, offset=by[:, :].offset,
                          ap=[[by[:, :].ap[0][0], 128], [2 * P, NT], [2, P], [1, 2]])
                cw_r = AP(tensor=cw.tensor, offset=cw[:, :].offset + 2 * side,
                          ap=[[cw[:, :].ap[0][0], 128], [4 * P, NT], [4, P], [1, 2]])
                nc.vector.tensor_tensor(out=cw_r, in0=bx_r, in1=by_v, op=Alu.mult)

            # kw[t, pt, rs, g] = cw[t, pt, rs] * wsm[t, pt, g]  (bf16)
            for rs in range(4):
                cw_rs = AP(tensor=cw.tensor, offset=cw[:, :].offset + rs,
                           ap=[[cw[:, :].ap[0][0], 128], [4 * P, NT], [4, P], [0, G]])
                w_v = AP(tensor=wsm.tensor, offset=wsm[:, :].offset,
                         ap=[[wsm[:, :].ap[0][0], 128], [P * G, NT], [G, P], [1, G]])
                kw_rs = AP(tensor=kw.tensor, offset=kw[:, :].offset + rs * G,
                           ap=[[kw[:, :].ap[0][0], 128], [4 * P * G, NT], [4 * G, P], [1, G]])
                nc.vector.tensor_tensor(out=kw_rs, in0=cw_rs, in1=w_v, op=Alu.mult)

            # sumcoef[t, g] = sum_pt wsm * (bx0+bx1)*(by0+by1)   (for value_b)
            bsx = bpool.tile([128, NP_], f32)
            bsy = bpool.tile([128, NP_], f32)
            bx0r = AP(tensor=bx.tensor, offset=bx[:, :].offset, ap=[[bx[:, :].ap[0][0], 128], [2, NP_]])
            bx1r = AP(tensor=bx.tensor, offset=bx[:, :].offset + 1, ap=[[bx[:, :].ap[0][0], 128], [2, NP_]])
            by0r = AP(tensor=by.tensor, offset=by[:, :].offset, ap=[[by[:, :].ap[0][0], 128], [2, NP_]])
            by1r = AP(tensor=by.tensor, offset=by[:, :].offset + 1, ap=[[by[:, :].ap[0][0], 128], [2, NP_]])
            nc.vector.tensor_tensor(out=bsx[:], in0=bx0r, in1=bx1r, op=Alu.add)
            nc.vector.tensor_tensor(out=bsy[:], in0=by0r, in1=by1r, op=Alu.add)
            bws = bpool.tile([128, NP_], bf16)
            nc.vector.tensor_tensor(out=bws[:], in0=bsx[:], in1=bsy[:], op=Alu.mult)
            wp = bpool.tile([128, NT * P * G], bf16)
            bws_b = AP(tensor=bws.tensor, offset=bws[:, :].offset,
                       ap=[[bws[:, :].ap[0][0], 128], [P, NT], [1, P], [0, G]])
            nc.vector.tensor_tensor(
                out=wp[:, :].rearrange("p (t q g) -> p t q g", q=P, g=G),
                in0=wsm[:, :].rearrange("p (t q g) -> p t q g", q=P, g=G),
                in1=bws_b, op=Alu.mult)
            wp_gp = AP(tensor=wp.tensor, offset=wp[:, :].offset,
                       ap=[[wp[:, :].ap[0][0], 128], [P * G, NT], [1, G], [G, P]])
            nc.vector.tensor_reduce(out=sumcoef[:, :].rearrange("p (t g) -> p t g", g=G),
                                    in_=wp_gp, axis=Ax.X, op=Alu.add)

            # ---- phase B2: idx16 build (PE permutation, 3 chunks per PSUM tile) ----
            # flat gather order l = slot*128 + q (slot = local (tt, pt)):
            # idx16[q%16, 8*j' + qh] = idxf[16qh + q%16, j'] globally.
            NGRP = 5
            for grp in range(NGRP):
                jlo = grp * 54           # 3 chunks x 18 slots
                i16ps = pspool.tile([128, 54 * 8], f32, tag="i16", bufs=2, name=f"i16ps{grp}")
                for qh in range(8):
                    outap = AP(tensor=i16ps.tensor, offset=i16ps[:, :].offset + qh,
                               ap=[[i16ps[:, :].ap[0][0], 128], [8, 54]])
                    nc.tensor.matmul(outap, oneh_s[:, qh * 128 : (qh + 1) * 128],
                                     idxf[:, jlo : jlo + 54],
                                     start=True, stop=True)
                nc.vector.tensor_copy(
                    idx16[:, jlo * 8 : (jlo + 54) * 8], i16ps[:])

            btmp_ctx.__exit__(None, None, None)

            # ---- phase C: gather + combine ----
            scr_src = AP(tensor=scr2.tensor, offset=scr2[:, :].offset,
                         ap=[[2 * C, NROW - 1], [1, ELEM]])

            with (
                tc.tile_pool(name="g", bufs=5) as gpool,
                tc.tile_pool(name="tree", bufs=2) as tpool,
                tc.tile_pool(name="aggp", bufs=2) as apool,
            ):
                n_ch = NCH if stage == "full" else int(stage)
                for ch in range(n_ch):
                    gt = gpool.tile([128, TCH * NJ, ELEM], bf16, tag="g", name=f"g{ch}")
                    nc.gpsimd.dma_gather(
                        gt[:, :, :], scr_src,
                        idx16[:, ch * (NIDX_CH // 16) : (ch + 1) * (NIDX_CH // 16)],
                        num_idxs=NIDX_CH, num_idxs_reg=NIDX_CH,
                        elem_size=ELEM, elem_step=2 * C, single_packet=False,
                        queue_num=ch % 4,
                    )

                    aggT2 = apool.tile([128, TCH * 128], bf16, tag="aggT", name=f"aggT{ch}")
                    for tt_ in range(TCH):
                        t = ch * TCH + tt_
                        gof = gt[:, :, :].offset + tt_ * NJ * ELEM
                        # kw16[q, (j, g, gc)] = kw[q, (j, g)] broadcast over gc
                        # (on ACT so the DVE multiply below runs in 2x mode)
                        kw16 = tpool.tile([128, 36 * C], bf16, tag="kw16", name=f"kw16_{t}")
                        kwb = AP(tensor=kw.tensor, offset=kw[:, :].offset + t * P * 4 * G,
                                 ap=[[kw[:, :].ap[0][0], 128], [G, 36], [1, G], [0, GC]])
                        k16v = AP(tensor=kw16.tensor, offset=kw16[:, :].offset,
                                  ap=[[kw16[:, :].ap[0][0], 128], [C, 36], [GC, G], [1, GC]])
                        nc.scalar.activation(k16v, kwb, Act.Copy)

                        # tp = gt * kw16  (all-contiguous bf16 -> DVE 2x)
                        tp = tpool.tile([128, 36 * C], bf16, tag="tp", name=f"tp{t}")
                        g_v = AP(tensor=gt.tensor, offset=gof,
                                 ap=[[gt[:, :, :].ap[0][0], 128], [1, 36 * C]])
                        nc.vector.tensor_tensor(out=tp[:], in0=g_v, in1=kw16[:], op=Alu.mult)

                        # pairwise tree reduce over the 36 corner blocks (2x mode)
                        def pair_add(dst, dof, src, sof, nblk):
                            i0 = AP(tensor=src.tensor, offset=src[:, :].offset + sof,
                                    ap=[[src[:, :].ap[0][0], 128], [2 * C, nblk], [1, C]])
                            i1 = AP(tensor=src.tensor, offset=src[:, :].offset + sof + C,
                                    ap=[[src[:, :].ap[0][0], 128], [2 * C, nblk], [1, C]])
                            o = AP(tensor=dst.tensor, offset=dst[:, :].offset + dof,
                                   ap=[[dst[:, :].ap[0][0], 128], [C, nblk], [1, C]])
                            nc.vector.tensor_tensor(out=o, in0=i0, in1=i1, op=Alu.add)

                        t1 = tpool.tile([128, 18 * C], bf16, tag="t1", name=f"t1_{t}")
                        pair_add(t1, 0, tp, 0, 18)
                        t2 = tp     # ping-pong: tp is dead after t1
                        pair_add(t2, 0, t1, 0, 9)
                        t3 = t1
                        pair_add(t3, 0, t2, 0, 4)
                        t4 = tpool.tile([128, 2 * C], bf16, tag="t4", name=f"t4_{t}")
                        pair_add(t4, 0, t3, 0, 2)
                        t5 = tpool.tile([128, C], bf16, tag="t5", name=f"t5_{t}")
                        pair_add(t5, 0, t4, 0, 1)

                        # ebias = value_b * sumcoef (per query, per group)
                        ebias = apool.tile([128, C], f32, tag="eb", name=f"eb{t}")
                        sc_v = AP(tensor=sumcoef.tensor, offset=sumcoef[:, :].offset + t * G,
                                  ap=[[sumcoef[:, :].ap[0][0], 128], [1, G], [0, GC]])
                        bv_v = bvr_s[:, :].rearrange("p (g c) -> p g c", g=G)
                        nc.vector.tensor_tensor(out=ebias[:, :].rearrange("p (g c) -> p g c", g=G),
                                                in0=sc_v, in1=bv_v, op=Alu.mult)
                        # agg = t5 + t2[block 8] + ebias
                        agg = apool.tile([128, C], bf16, tag="agg", name=f"agg{t}")
                        t2tail = AP(tensor=t2.tensor, offset=t2[:, :].offset + 8 * C,
                                    ap=[[t2[:, :].ap[0][0], 128], [1, C]])
                        nc.vector.tensor_tensor(out=agg[:], in0=t5[:], in1=t2tail, op=Alu.add)
                        agg2 = apool.tile([128, C], bf16, tag="agg2", name=f"agg2{t}")
                        nc.vector.tensor_tensor(out=agg2[:], in0=agg[:], in1=ebias[:], op=Alu.add)

                        # transpose -> [c, q] (bf16)
                        trps = pspool.tile([128, C], bf16, tag="tr", bufs=1, name=f"tr{t}")
                        nc.tensor.transpose(trps[:], agg2[:], ident_s[:])
                        nc.scalar.copy(aggT2[:, tt_ * 128 : (tt_ + 1) * 128], trps[:])

                    # batched out-projection for the chunk: [c_out, 256]
                    fops = pspool.tile([128, TCH * 128], f32, tag="fo", bufs=1, name=f"fo{ch}")
                    nc.tensor.matmul(fops[:], owTb_s[:], aggT2[:], start=True, stop=True)
                    fo_sb = apool.tile([128, TCH * 128], f32, tag="fosb", name=f"fosb{ch}")
                    nc.scalar.activation(fo_sb[:], fops[:], Act.Identity,
                                         bias=outb_s[:, 0:1], scale=1.0)
                    nc.scalar.dma_start(out_d[:, ch * TCH * 128 : (ch + 1) * TCH * 128], fo_sb[:])

    nc.finalize()
    return nc


def _host_prep(inputs):
    """Prepare per-core input maps from full inputs."""
    feats = np.asarray(inputs["feats"], np.float32)          # [B, C, H, W]
    anchor = np.asarray(inputs["anchor_points"], np.float32)  # [B, HW, 2]
    value_w = np.asarray(inputs["value_w"], np.float32)
    value_b = np.asarray(inputs["value_b"], np.float32)
    weights_w = np.asarray(inputs["weights_w"], np.float32)
    weights_b = np.asarray(inputs["weights_b"], np.float32)
    offset_w = np.asarray(inputs["offset_w"], np.float32)
    offset_b = np.asarray(inputs["offset_b"], np.float32)
    out_w = np.asarray(inputs["out_w"], np.float32)
    out_b = np.asarray(inputs["out_b"], np.float32)

    w90 = np.concatenate([weights_w, offset_w], 0)            # [90, C]
    b90 = np.concatenate([weights_b, offset_b], 0)            # [90]
    shared = {
        "vwTb": np.ascontiguousarray(value_w.T).astype(ml_dtypes.bfloat16),
        "w90T": np.ascontiguousarray(w90.T),
        "owTb": np.ascontiguousarray(out_w.T).astype(ml_dtypes.bfloat16),
        "b90r": np.broadcast_to(b90, (128, 90)).copy(),
        "bvr": np.broadcast_to(value_b, (128, C)).copy(),
        "outb": out_b.reshape(128, 1).copy(),
        "ident": np.eye(128, dtype=ml_dtypes.bfloat16),
    }
    oneh = np.zeros((128, 8, 128), np.float32)
    for qh in range(8):
        for m in range(128):
            oneh[16 * qh + (m % 16), qh, m] = 1.0
    shared["oneh"] = oneh.reshape(128, 8 * 128)

    in_maps = []
    for core in range(NCORES):
        b_i, sl = core // 4, core % 4
        off = sl * QS
        fr = np.roll(feats[b_i].reshape(C, HW), -off, axis=1)
        fx = np.concatenate([fr, fr[:, : FVLEN - HW]], axis=1)
        an = anchor[b_i, off : off + QS].reshape(NT, 128, 2).transpose(1, 0, 2).reshape(128, NT * 2)
        m = dict(shared)
        m["fvalb"] = np.ascontiguousarray(fx).astype(ml_dtypes.bfloat16)
        m["fproj"] = np.ascontiguousarray(fr[:, :QS])
        m["anch"] = np.ascontiguousarray(an)
        m["rotoff"] = np.full((128, 1), float(off), np.float32)
        in_maps.append(m)
    return in_maps


def kernel(**inputs) -> np.ndarray:
    from concourse.bass_utils import run_bass_kernel_spmd

    if "nc" not in _CACHE:
        _CACHE["nc"] = _build_nc()
    nc = _CACHE["nc"]
    in_maps = _host_prep(inputs)
    res = run_bass_kernel_spmd(nc, in_maps, core_ids=list(range(NCORES)))
    out = np.zeros((B, C, HW), np.float32)
    for core in range(NCORES):
        b_i, sl = core // 4, core % 4
        out[b_i, :, sl * QS : (sl + 1) * QS] = res.results[core]["out"]
    return out.reshape(B, C, H, W)


# revision 14
# speedup vs baseline: 1.8342x; 1.0586x over previous
"""Deformable 2D feature aggregator — Trainium2 Bass kernel, 8-core SPMD. v2.

Problem: B=2, C=128, H=96, W=160, P=9 points, G=8 groups.
  value = conv1x1(feats); w = softmax over P of conv1x1(feats); offs = conv1x1(feats)
  pts = anchors + offs; out_proj(conv-weighted bilinear gather of value at pts).

Sharding: 8 cores = 2 batches x 4 query-slices, rotated pixel ring per core.

v3 design:
  - Vertical-pair DRAM scratch scr2[r] = [v(r), v(r+W)] (512B rows). A single
    dma_gather element of 1KB with elem_step=512B starting at row (y0*W+x0)
    covers the full 2x2 bilinear stencil -> ONE gather index per (query,
    point), quartering v1's Q7 descriptor-generation time.
  - dma_gather spread over all 4 SWDGE queues (each = its own Q7 core pair)
    so descriptor generation overlaps up to 4x.
  - Value projection bf16 channel-major (one stationary LDWEIGHTS, 512-wide
    moving), two strided-output xbar transpose-DMAs interleave the (0, +W)
    shifts into SBUF, one SWDGE DMA (0.34ns/desc) writes scr2. f32 projection
    for offsets/weights.
  - Combine: ACT pre-broadcasts kw over gc (kw16) so the DVE multiply runs in
    2x mode; contiguous pairwise tree-reduce in bf16.
"""
import sys

sys.path.insert(0, "/opt/trn_rl_repo")

import numpy as np
import ml_dtypes

import concourse.bass as bass
import concourse.bacc as bacc
import concourse.mybir as mybir
import concourse.tile as tile
from concourse.ap import AP

# problem constants (hardcoded per harness contract)
B, C, H, W = 2, 128, 96, 160
HW = H * W                     # 15360
P, G, GC = 9, 8, 16
NCORES = 8
QS = B * HW // NCORES          # 3840 queries per core
NT = QS // 128                 # 30 query tiles
TCH = 2                        # query tiles per gather chunk
NCH = NT // TCH                # 15 gather chunks
NJ = P                         # 9 stencil gathers per query
NIDX_CH = TCH * 128 * NJ       # 2304 gather indices per chunk
SHIFT = 1024.0                 # floor-bias (exact in f32 for our range)
NPXT = HW // 128               # 120 pixel tiles
NROW = HW + 256                # scr2 rows (pad tiles back idx+1 and the +W copy)
FVLEN = 16384                  # fvalb padded length (wrap cols appended)
NVMM = FVLEN // 512            # 32 value matmuls
FPCH = 640                     # f32 proj feats chunk
ELEM = 4 * C                   # gather elem: 4 corners x C (bf16)

f32 = mybir.dt.float32
bf16 = mybir.dt.bfloat16
i16 = mybir.dt.int16
Alu = mybir.AluOpType
Act = mybir.ActivationFunctionType
Ax = mybir.AxisListType

_CACHE: dict = {}


def _build_nc(stage=None):
    import os
    stage = stage or os.environ.get("BASS_STAGE", "full")
    nc = bacc.Bacc(num_swdge_queues=4)

    fvalb = nc.dram_tensor("fvalb", [C, FVLEN], bf16, kind="ExternalInput")
    fproj = nc.dram_tensor("fproj", [C, QS], f32, kind="ExternalInput")
    anch = nc.dram_tensor("anch", [128, NT * 2], f32, kind="ExternalInput")
    vwTb = nc.dram_tensor("vwTb", [C, C], bf16, kind="ExternalInput")
    w90T = nc.dram_tensor("w90T", [C, 90], f32, kind="ExternalInput")
    owTb = nc.dram_tensor("owTb", [C, C], bf16, kind="ExternalInput")
    b90r = nc.dram_tensor("b90r", [128, 90], f32, kind="ExternalInput")
    bvr = nc.dram_tensor("bvr", [128, C], f32, kind="ExternalInput")
    outb = nc.dram_tensor("outb", [128, 1], f32, kind="ExternalInput")
    oneh = nc.dram_tensor("oneh", [128, 8 * 128], f32, kind="ExternalInput")
    ident = nc.dram_tensor("ident", [128, 128], bf16, kind="ExternalInput")
    rotoff = nc.dram_tensor("rotoff", [128, 1], f32, kind="ExternalInput")
    out_d = nc.dram_tensor("out", [C, QS], f32, kind="ExternalOutput")

    with tile.TileContext(nc) as tc, nc.allow_low_precision("bf16 combine by design"):
        with (
            tc.tile_pool(name="const", bufs=1) as cpool,
            tc.tile_pool(name="stage", bufs=1) as spool,
            tc.tile_pool(name="ps", bufs=1, space="PSUM") as pspool,
            tc.tile_pool(name="dram", bufs=1, space="DRAM") as dpool,
        ):
            # ---- persistent loads ----
            vwTb_s = cpool.tile([C, C], bf16)
            nc.sync.dma_start(vwTb_s[:], vwTb[:])
            w90T_s = cpool.tile([C, 90], f32)
            nc.sync.dma_start(w90T_s[:], w90T[:])
            owTb_s = cpool.tile([C, C], bf16)
            nc.sync.dma_start(owTb_s[:], owTb[:])
            b90_s = cpool.tile([128, 90], f32)
            nc.sync.dma_start(b90_s[:], b90r[:])
            bvr_s = cpool.tile([128, C], f32)
            nc.sync.dma_start(bvr_s[:], bvr[:])
            outb_s = cpool.tile([128, 1], f32)
            nc.sync.dma_start(outb_s[:], outb[:])
            oneh_s = cpool.tile([128, 8 * 128], f32)
            nc.sync.dma_start(oneh_s[:], oneh[:])
            ident_s = cpool.tile([128, 128], bf16)
            nc.sync.dma_start(ident_s[:], ident[:])
            anch_s = cpool.tile([128, NT * 2], f32)
            nc.sync.dma_start(anch_s[:], anch[:])
            rot_s = cpool.tile([128, 1], f32)
            nc.sync.dma_start(rot_s[:], rotoff[:])

            # vertical-pair scratch: row r = [v(r), v(r+W)] of the ring
            # (256 bf16 = 512B). A 1KB gather elem at step 512B from row r
            # yields corners [(r),(r+W),(r+1),(r+1+W)]. Rows beyond the valid
            # token range hold wrap junk (finite, never gathered).
            scr2 = dpool.tile([NROW, 2 * C], bf16)

            # whole-kernel staging tiles
            kw = spool.tile([128, NT * P * 4 * G], bf16)
            sumcoef = spool.tile([128, NT * G], f32)
            idx16 = spool.tile([128, NCH * (NIDX_CH // 16)], i16)

            # ---- phase A: value map (channel-major bf16) -> scr4 ----
            btmp_ctx = tc.tile_pool(name="btmp", bufs=1)
            bpool = btmp_ctx.__enter__()
            proj_s = bpool.tile([128, NT * 90], f32)
            with (
                tc.tile_pool(name="vmapp", bufs=1) as vmpool,
                tc.tile_pool(name="fb", bufs=2) as fbpool,
                tc.tile_pool(name="fp", bufs=2) as fppool,
            ):
                vmap = vmpool.tile([128, FVLEN], bf16)
                for m in range(NVMM):
                    if m % 4 == 0:
                        fbch = fbpool.tile([128, 2048], bf16, tag="fb", name=f"fb{m}")
                        nc.scalar.dma_start(fbch[:], fvalb[:, m * 512 : m * 512 + 2048])
                    colb = (m % 4) * 512
                    vps = pspool.tile([128, 512], f32, tag="v", bufs=2, name=f"vps{m}")
                    nc.tensor.matmul(vps[:], vwTb_s[:], fbch[:, colb : colb + 512],
                                     start=True, stop=True)
                    if m % 2 == 0:
                        nc.scalar.copy(vmap[:, m * 512 : (m + 1) * 512], vps[:])
                    else:
                        nc.vector.tensor_copy(vmap[:, m * 512 : (m + 1) * 512], vps[:])

                # staging (slot 0 only): stokq[p, t, c] = vmap[c, 128t + p],
                # quartered xbar transposes overlapping the matmul chain, each
                # followed by a SWDGE-generated write of scr2's slot-0 column
                # (256B interleaved descriptors; Q7 CounterMachine ~0.34ns/desc).
                # Slot 1 (the +W row) is then derived with row-shifted
                # DRAM->DRAM SWDGE copies instead of a second transpose pass,
                # halving the head's fabric traffic.
                NTT = NROW // 128
                QT = (NTT + 3) // 4
                qbounds = []
                for qi, t0 in enumerate(range(0, NTT, QT)):
                    nt = min(QT, NTT - t0)
                    qbounds.append((t0, nt))
                    stokq = fbpool.tile([128, QT, C], bf16, tag="stok", bufs=3, name=f"stok{qi}")
                    o = AP(tensor=stokq.tensor, offset=stokq[:, :, :].offset,
                           ap=[[stokq[:, :, :].ap[0][0], 128], [C, nt], [1, C]])
                    nc.sync.dma_start_transpose(
                        o, vmap[:, t0 * 128 : t0 * 128 + nt * 128])
                    i_ap = AP(tensor=stokq.tensor, offset=stokq[:, :, :].offset,
                              ap=[[stokq[:, :, :].ap[0][0], 128], [1, nt * C]])
                    o = AP(tensor=scr2.tensor, offset=scr2[:, :].offset + t0 * 128 * 2 * C,
                           ap=[[2 * C, 128], [128 * 2 * C, nt], [1, C]])
                    nc.gpsimd.dma_start(o, i_ap)
                # slot1[r] = slot0[r + W]; quarter k reads rows written by
                # quarter k and the first tiles of k+1, so emit after both.
                for qi, (t0, nt) in enumerate(qbounds):
                    r0, n = t0 * 128, nt * 128
                    if r0 + n + W > NROW:
                        n = NROW - W - r0
                    if n <= 0:
                        continue
                    i_ap = AP(tensor=scr2.tensor,
                              offset=scr2[:, :].offset + (r0 + W) * 2 * C,
                              ap=[[2 * C, n], [1, C]])
                    o = AP(tensor=scr2.tensor,
                           offset=scr2[:, :].offset + r0 * 2 * C + C,
                           ap=[[2 * C, n], [1, C]])
                    nc.gpsimd.dma_start(o, i_ap)

                # ---- f32 projection (weights/offsets) for this core's queries ----
                for t in range(NT):
                    if t % (FPCH // 128) == 0:
                        fpch = fppool.tile([128, FPCH], f32, tag="fp", name=f"fp{t}")
                        nc.scalar.dma_start(fpch[:], fproj[:, t * 128 : t * 128 + FPCH])
                    col = (t % (FPCH // 128)) * 128
                    pps = pspool.tile([128, 90], f32, tag="p", bufs=2, name=f"pps{t}")
                    nc.tensor.matmul(pps[:], fpch[:, col : col + 128], w90T_s[:],
                                     start=True, stop=True)
                    nc.vector.tensor_tensor(
                        out=proj_s[:, t * 90 : (t + 1) * 90],
                        in0=pps[:],
                        in1=b90_s[:],
                        op=Alu.add,
                    )

            # ---- phase B: batched softmax / coords / weights (query-major) ----
            # proj_s free layout per tile t: [0,72) = wlog (pt*8+g), [72,90) = offs (pt*2+xy)
            wmax = bpool.tile([128, NT * G], f32)
            wl_gp = AP(tensor=proj_s.tensor, offset=proj_s[:, :].offset,
                       ap=[[proj_s[:, :].ap[0][0], 128], [90, NT], [1, G], [G, P]])
            nc.vector.tensor_reduce(out=wmax[:, :].rearrange("p (t g) -> p t g", g=G),
                                    in_=wl_gp, axis=Ax.X, op=Alu.max)
            smf = bpool.tile([128, NT * P * G], f32)
            wl_pg = AP(tensor=proj_s.tensor, offset=proj_s[:, :].offset,
                       ap=[[proj_s[:, :].ap[0][0], 128], [90, NT], [G, P], [1, G]])
            wmax_b = AP(tensor=wmax.tensor, offset=wmax[:, :].offset,
                        ap=[[wmax[:, :].ap[0][0], 128], [G, NT], [0, P], [1, G]])
            nc.vector.tensor_tensor(
                out=smf[:, :].rearrange("p (t q g) -> p t q g", q=P, g=G),
                in0=wl_pg, in1=wmax_b, op=Alu.subtract)
            nc.scalar.activation(smf[:], smf[:], Act.Exp)
            ssum = bpool.tile([128, NT * G], f32)
            sm_gp = AP(tensor=smf.tensor, offset=smf[:, :].offset,
                       ap=[[smf[:, :].ap[0][0], 128], [P * G, NT], [1, G], [G, P]])
            nc.vector.tensor_reduce(out=ssum[:, :].rearrange("p (t g) -> p t g", g=G),
                                    in_=sm_gp, axis=Ax.X, op=Alu.add)
            rcps = bpool.tile([128, NT * G], f32)
            nc.vector.reciprocal(rcps[:], ssum[:])
            wsm = bpool.tile([128, NT * P * G], bf16)
            rcp_b = AP(tensor=rcps.tensor, offset=rcps[:, :].offset,
                       ap=[[rcps[:, :].ap[0][0], 128], [G, NT], [0, P], [1, G]])
            nc.vector.tensor_tensor(
                out=wsm[:, :].rearrange("p (t q g) -> p t q g", q=P, g=G),
                in0=smf[:, :].rearrange("p (t q g) -> p t q g", q=P, g=G),
                in1=rcp_b, op=Alu.mult)

            # coords: px/py [128, NT*P] laid out (t, pt)
            NP_ = NT * P

            px = bpool.tile([128, NP_], f32)
            py = bpool.tile([128, NP_], f32)
            offs_x = AP(tensor=proj_s.tensor, offset=proj_s[:, :].offset + 72,
                        ap=[[proj_s[:, :].ap[0][0], 128], [90, NT], [2, P]])
            offs_y = AP(tensor=proj_s.tensor, offset=proj_s[:, :].offset + 73,
                        ap=[[proj_s[:, :].ap[0][0], 128], [90, NT], [2, P]])
            anx = AP(tensor=anch_s.tensor, offset=anch_s[:, :].offset,
                     ap=[[anch_s[:, :].ap[0][0], 128], [2, NT], [0, P]])
            any_ = AP(tensor=anch_s.tensor, offset=anch_s[:, :].offset + 1,
                      ap=[[anch_s[:, :].ap[0][0], 128], [2, NT], [0, P]])
            pxv = px[:, :].rearrange("p (t q) -> p t q", q=P)
            pyv = py[:, :].rearrange("p (t q) -> p t q", q=P)
            nc.vector.tensor_tensor(out=pxv, in0=offs_x, in1=anx, op=Alu.add)
            nc.vector.tensor_tensor(out=pyv, in0=offs_y, in1=any_, op=Alu.add)

            xp = bpool.tile([128, NP_], f32)
            yp = bpool.tile([128, NP_], f32)
            nc.scalar.activation(xp[:], px[:], Act.Copy, bias=SHIFT - 0.5, scale=float(W))
            nc.scalar.activation(yp[:], py[:], Act.Copy, bias=SHIFT - 0.5, scale=float(H))
            # floor via round(x-0.5): (x + (2^23-0.5)) - 2^23. At integer x the
            # half-even tie may floor one low with frac 1.0 — an equivalent
            # bilinear weighting, so interpolation is unchanged.
            MAGIC = float(1 << 23)
            xf = bpool.tile([128, NP_], f32)
            yf = bpool.tile([128, NP_], f32)
            nc.vector.tensor_scalar(out=xf[:], in0=xp[:], scalar1=MAGIC - 0.5,
                                    scalar2=MAGIC, op0=Alu.add, op1=Alu.subtract)
            nc.vector.tensor_scalar(out=yf[:], in0=yp[:], scalar1=MAGIC - 0.5,
                                    scalar2=MAGIC, op0=Alu.add, op1=Alu.subtract)
            wx = bpool.tile([128, NP_], f32)
            wy = bpool.tile([128, NP_], f32)
            nc.vector.tensor_tensor(out=wx[:], in0=xp[:], in1=xf[:], op=Alu.subtract)
            nc.vector.tensor_tensor(out=wy[:], in0=yp[:], in1=yf[:], op=Alu.subtract)

            # token coords, clamped: x in [0, W-2], y in [0, H-2]
            xg = bpool.tile([128, NP_], f32)
            nc.vector.tensor_scalar(out=xg[:], in0=xf[:], scalar1=SHIFT, scalar2=0.0,
                                    op0=Alu.subtract, op1=Alu.max)
            nc.vector.tensor_scalar(out=xg[:], in0=xg[:], scalar1=float(W - 2), scalar2=None, op0=Alu.min)
            yg = bpool.tile([128, NP_], f32)
            nc.vector.tensor_scalar(out=yg[:], in0=yf[:], scalar1=SHIFT, scalar2=0.0,
                                    op0=Alu.subtract, op1=Alu.max)
            nc.vector.tensor_scalar(out=yg[:], in0=yg[:], scalar1=float(H - 2), scalar2=None, op0=Alu.min)

            ux = bpool.tile([128, NP_], f32)
            uy = bpool.tile([128, NP_], f32)
            nc.scalar.activation(ux[:], wx[:], Act.Copy, bias=1.0, scale=-1.0)
            nc.scalar.activation(uy[:], wy[:], Act.Copy, bias=1.0, scale=-1.0)

            # validity masks with edge-clamp weight swap (x and y symmetric):
            # b0 = u*mA + w*mB ; b1 = w*mA + u*mC
            #   mA = [0 <= f <= L-2], mB = [f == -1], mC = [f == L-1]
            tA = bpool.tile([128, NP_], f32)
            tB = bpool.tile([128, NP_], f32)
            v1 = bpool.tile([128, NP_], f32)
            v2 = bpool.tile([128, NP_], f32)

            def edge_weights(bx, f, w_, u_, L):
                mA = bpool.tile([128, NP_], f32)
                nc.vector.tensor_scalar(out=tA[:], in0=f[:], scalar1=SHIFT, scalar2=None, op0=Alu.is_ge)
                nc.vector.tensor_scalar(out=tB[:], in0=f[:], scalar1=SHIFT + L - 2, scalar2=None, op0=Alu.is_le)
                nc.vector.tensor_tensor(out=mA[:], in0=tA[:], in1=tB[:], op=Alu.mult)
                mB = bpool.tile([128, NP_], f32)
                nc.vector.tensor_scalar(out=mB[:], in0=f[:], scalar1=SHIFT - 1.0, scalar2=None, op0=Alu.is_equal)
                mC = bpool.tile([128, NP_], f32)
                nc.vector.tensor_scalar(out=mC[:], in0=f[:], scalar1=SHIFT + L - 1, scalar2=None, op0=Alu.is_equal)
                b0 = AP(tensor=bx.tensor, offset=bx[:, :].offset,
                        ap=[[bx[:, :].ap[0][0], 128], [2, NP_]])
                b1 = AP(tensor=bx.tensor, offset=bx[:, :].offset + 1,
                        ap=[[bx[:, :].ap[0][0], 128], [2, NP_]])
                nc.vector.tensor_tensor(out=v1[:], in0=u_[:], in1=mA[:], op=Alu.mult)
                nc.vector.tensor_tensor(out=v2[:], in0=w_[:], in1=mB[:], op=Alu.mult)
                nc.vector.tensor_tensor(out=b0, in0=v1[:], in1=v2[:], op=Alu.add)
                nc.vector.tensor_tensor(out=v1[:], in0=w_[:], in1=mA[:], op=Alu.mult)
                nc.vector.tensor_tensor(out=v2[:], in0=u_[:], in1=mC[:], op=Alu.mult)
                nc.vector.tensor_tensor(out=b1, in0=v1[:], in1=v2[:], op=Alu.add)

            bx = bpool.tile([128, NP_ * 2], f32)   # (t, pt, side)
            by = bpool.tile([128, NP_ * 2], f32)   # (t, pt, row)
            edge_weights(bx, xf, wx, ux, W)
            edge_weights(by, yf, wy, uy, H)

            # gather supertoken indices (rotated): idx = (yg*W + xg - rotoff) mod HW
            idxf = bpool.tile([128, NP_], f32)    # (t, pt)
            r0t = bpool.tile([128, NP_], f32)
            nc.scalar.activation(r0t[:], yg[:], Act.Copy, bias=0.0, scale=float(W))
            nc.vector.tensor_tensor(out=idxf[:], in0=r0t[:], in1=xg[:], op=Alu.add)
            nc.vector.tensor_scalar(out=idxf[:], in0=idxf[:], scalar1=rot_s[:, 0:1],
                                    scalar2=None, op0=Alu.subtract)
            wrap = bpool.tile([128, NP_], f32)
            nc.vector.tensor_scalar(out=wrap[:], in0=idxf[:], scalar1=0.0,
                                    scalar2=float(HW), op0=Alu.is_lt, op1=Alu.mult)
            nc.vector.tensor_tensor(out=idxf[:], in0=idxf[:], in1=wrap[:], op=Alu.add)

            # cw[t, pt, side, row] = bx[t,pt,side] * by[t,pt,row]  (bf16)
            # (gather elem corner order is side-major: r, r+W, r+1, r+1+W)
            cw = bpool.tile([128, NT * P * 4], bf16)
            for side in range(2):
                bx_r = AP(tensor=bx.tensor, offset=bx[:, :].offset + side,
                          ap=[[bx[:, :].ap[0][0], 128], [2 * P, NT], [2, P], [0, 2]])
                by_v = AP(tensor=by.tensor, offset=by[:, :].offset,
                          ap=[[by[:, :].ap[0][0], 128], [2 * P, NT], [2, P], [1, 2]])
                cw_r = AP(tensor=cw.tensor, offset=cw[:, :].offset + 2 * side,
                          ap=[[cw[:, :].ap[0][0], 128], [4 * P, NT], [4, P], [1, 2]])
                nc.vector.tensor_tensor(out=cw_r, in0=bx_r, in1=by_v, op=Alu.mult)

            # kw[t, pt, rs, g] = cw[t, pt, rs] * wsm[t, pt, g]  (bf16)
            for rs in range(4):
                cw_rs = AP(tensor=cw.tensor, offset=cw[:, :].offset + rs,
                           ap=[[cw[:, :].ap[0][0], 128], [4 * P, NT], [4, P], [0, G]])
                w_v = AP(tensor=wsm.tensor, offset=wsm[:, :].offset,
                         ap=[[wsm[:, :].ap[0][0], 128], [P * G, NT], [G, P], [1, G]])
                kw_rs = AP(tensor=kw.tensor, offset=kw[:, :].offset + rs * G,
                           ap=[[kw[:, :].ap[0][0], 128], [4 * P * G, NT], [4 * G, P], [1, G]])
                nc.vector.tensor_tensor(out=kw_rs, in0=cw_rs, in1=w_v, op=Alu.mult)

            # sumcoef[t, g] = sum_pt wsm * (bx0+bx1)*(by0+by1)   (for value_b)
            bsx = bpool.tile([128, NP_], f32)
            bsy = bpool.tile([128, NP_], f32)
            bx0r = AP(tensor=bx.tensor, offset=bx[:, :].offset, ap=[[bx[:, :].ap[0][0], 128], [2, NP_]])
            bx1r = AP(tensor=bx.tensor, offset=bx[:, :].offset + 1, ap=[[bx[:, :].ap[0][0], 128], [2, NP_]])
            by0r = AP(tensor=by.tensor, offset=by[:, :].offset, ap=[[by[:, :].ap[0][0], 128], [2, NP_]])
            by1r = AP(tensor=by.tensor, offset=by[:, :].offset + 1, ap=[[by[:, :].ap[0][0], 128], [2, NP_]])
            nc.vector.tensor_tensor(out=bsx[:], in0=bx0r, in1=bx1r, op=Alu.add)
            nc.vector.tensor_tensor(out=bsy[:], in0=by0r, in1=by1r, op=Alu.add)
            bws = bpool.tile([128, NP_], bf16)
            nc.vector.tensor_tensor(out=bws[:], in0=bsx[:], in1=bsy[:], op=Alu.mult)
            wp = bpool.tile([128, NT * P * G], bf16)
            bws_b = AP(tensor=bws.tensor, offset=bws[:, :].offset,
                       ap=[[bws[:, :].ap[0][0], 128], [P, NT], [1, P], [0, G]])
            nc.vector.tensor_tensor(
                out=wp[:, :].rearrange("p (t q g) -> p t q g", q=P, g=G),
                in0=wsm[:, :].rearrange("p (t q g) -> p t q g", q=P, g=G),
                in1=bws_b, op=Alu.mult)
            wp_gp = AP(tensor=wp.tensor, offset=wp[:, :].offset,
                       ap=[[wp[:, :].ap[0][0], 128], [P * G, NT], [1, G], [G, P]])
            nc.vector.tensor_reduce(out=sumcoef[:, :].rearrange("p (t g) -> p t g", g=G),
                                    in_=wp_gp, axis=Ax.X, op=Alu.add)

            # ---- phase B2: idx16 build (PE permutation, 3 chunks per PSUM tile) ----
            # flat gather order l = slot*128 + q (slot = local (tt, pt)):
            # idx16[q%16, 8*j' + qh] = idxf[16qh + q%16, j'] globally.
            NGRP = 5
            for grp in range(NGRP):
                jlo = grp * 54           # 3 chunks x 18 slots
                i16ps = pspool.tile([128, 54 * 8], f32, tag="i16", bufs=2, name=f"i16ps{grp}")
                for qh in range(8):
                    outap = AP(tensor=i16ps.tensor, offset=i16ps[:, :].offset + qh,
                               ap=[[i16ps[:, :].ap[0][0], 128], [8, 54]])
                    nc.tensor.matmul(outap, oneh_s[:, qh * 128 : (qh + 1) * 128],
                                     idxf[:, jlo : jlo + 54],
                                     start=True, stop=True)
                nc.vector.tensor_copy(
                    idx16[:, jlo * 8 : (jlo + 54) * 8], i16ps[:])

            btmp_ctx.__exit__(None, None, None)

            # ---- phase C: gather + combine ----
            scr_src = AP(tensor=scr2.tensor, offset=scr2[:, :].offset,
                         ap=[[2 * C, NROW - 1], [1, ELEM]])

            with (
                tc.tile_pool(name="g", bufs=5) as gpool,
                tc.tile_pool(name="tree", bufs=2) as tpool,
                tc.tile_pool(name="aggp", bufs=2) as apool,
            ):
                n_ch = NCH if stage == "full" else int(stage)
                for ch in range(n_ch):
                    gt = gpool.tile([128, TCH * NJ, ELEM], bf16, tag="g", name=f"g{ch}")
                    nc.gpsimd.dma_gather(
                        gt[:, :, :], scr_src,
                        idx16[:, ch * (NIDX_CH // 16) : (ch + 1) * (NIDX_CH // 16)],
                        num_idxs=NIDX_CH, num_idxs_reg=NIDX_CH,
                        elem_size=ELEM, elem_step=2 * C, single_packet=False,
                        queue_num=ch % 4,
                    )

                    aggT2 = apool.tile([128, TCH * 128], bf16, tag="aggT", name=f"aggT{ch}")
                    for tt_ in range(TCH):
                        t = ch * TCH + tt_
                        gof = gt[:, :, :].offset + tt_ * NJ * ELEM
                        # kw16[q, (j, g, gc)] = kw[q, (j, g)] broadcast over gc
                        # (on ACT so the DVE multiply below runs in 2x mode)
                        kw16 = tpool.tile([128, 36 * C], bf16, tag="kw16", name=f"kw16_{t}")
                        kwb = AP(tensor=kw.tensor, offset=kw[:, :].offset + t * P * 4 * G,
                                 ap=[[kw[:, :].ap[0][0], 128], [G, 36], [1, G], [0, GC]])
                        k16v = AP(tensor=kw16.tensor, offset=kw16[:, :].offset,
                                  ap=[[kw16[:, :].ap[0][0], 128], [C, 36], [GC, G], [1, GC]])
                        nc.scalar.activation(k16v, kwb, Act.Copy)

                        # tp = gt * kw16  (all-contiguous bf16 -> DVE 2x)
                        tp = tpool.tile([128, 36 * C], bf16, tag="tp", name=f"tp{t}")
                        g_v = AP(tensor=gt.tensor, offset=gof,
                                 ap=[[gt[:, :, :].ap[0][0], 128], [1, 36 * C]])
                        nc.vector.tensor_tensor(out=tp[:], in0=g_v, in1=kw16[:], op=Alu.mult)

                        # pairwise tree reduce over the 36 corner blocks (2x mode)
                        def pair_add(dst, dof, src, sof, nblk):
                            i0 = AP(tensor=src.tensor, offset=src[:, :].offset + sof,
                                    ap=[[src[:, :].ap[0][0], 128], [2 * C, nblk], [1, C]])
                            i1 = AP(tensor=src.tensor, offset=src[:, :].offset + sof + C,
                                    ap=[[src[:, :].ap[0][0], 128], [2 * C, nblk], [1, C]])
                            o = AP(tensor=dst.tensor, offset=dst[:, :].offset + dof,
                                   ap=[[dst[:, :].ap[0][0], 128], [C, nblk], [1, C]])
                            nc.vector.tensor_tensor(out=o, in0=i0, in1=i1, op=Alu.add)

                        t1 = tpool.tile([128, 18 * C], bf16, tag="t1", name=f"t1_{t}")
                        pair_add(t1, 0, tp, 0, 18)
                        t2 = tp     # ping-pong: tp is dead after t1
                        pair_add(t2, 0, t1, 0, 9)
                        t3 = t1
                        pair_add(t3, 0, t2, 0, 4)
                        t4 = tpool.tile([128, 2 * C], bf16, tag="t4", name=f"t4_{t}")
                        pair_add(t4, 0, t3, 0, 2)
                        t5 = tpool.tile([128, C], bf16, tag="t5", name=f"t5_{t}")
                        pair_add(t5, 0, t4, 0, 1)

                        # ebias = value_b * sumcoef (per query, per group)
                        ebias = apool.tile([128, C], f32, tag="eb", name=f"eb{t}")
                        sc_v = AP(tensor=sumcoef.tensor, offset=sumcoef[:, :].offset + t * G,
                                  ap=[[sumcoef[:, :].ap[0][0], 128], [1, G], [0, GC]])
                        bv_v = bvr_s[:, :].rearrange("p (g c) -> p g c", g=G)
                        nc.vector.tensor_tensor(out=ebias[:, :].rearrange("p (g c) -> p g c", g=G),
                                                in0=sc_v, in1=bv_v, op=Alu.mult)
                        # agg = t5 + t2[block 8] + ebias
                        agg = apool.tile([128, C], bf16, tag="agg", name=f"agg{t}")
                        t2tail = AP(tensor=t2.tensor, offset=t2[:, :].offset + 8 * C,
                                    ap=[[t2[:, :].ap[0][0], 128], [1, C]])
                        nc.vector.tensor_tensor(out=agg[:], in0=t5[:], in1=t2tail, op=Alu.add)
                        agg2 = apool.tile([128, C], bf16, tag="agg2", name=f"agg2{t}")
                        nc.vector.tensor_tensor(out=agg2[:], in0=agg[:], in1=ebias[:], op=Alu.add)

                        # transpose -> [c, q] (bf16)
                        trps = pspool.tile([128, C], bf16, tag="tr", bufs=1, name=f"tr{t}")
                        nc.tensor.transpose(trps[:], agg2[:], ident_s[:])
                        nc.scalar.copy(aggT2[:, tt_ * 128 : (tt_ + 1) * 128], trps[:])

                    # batched out-projection for the chunk: [c_out, 256]
                    fops = pspool.tile([128, TCH * 128], f32, tag="fo", bufs=1, name=f"fo{ch}")
                    nc.tensor.matmul(fops[:], owTb_s[:], aggT2[:], start=True, stop=True)
                    fo_sb = apool.tile([128, TCH * 128], f32, tag="fosb", name=f"fosb{ch}")
                    nc.scalar.activation(fo_sb[:], fops[:], Act.Identity,
                                         bias=outb_s[:, 0:1], scale=1.0)
                    nc.scalar.dma_start(out_d[:, ch * TCH * 128 : (ch + 1) * TCH * 128], fo_sb[:])

    nc.finalize()
    return nc


def _host_prep(inputs):
    """Prepare per-core input maps from full inputs."""
    feats = np.asarray(inputs["feats"], np.float32)          # [B, C, H, W]
    anchor = np.asarray(inputs["anchor_points"], np.float32)  # [B, HW, 2]
    value_w = np.asarray(inputs["value_w"], np.float32)
    value_b = np.asarray(inputs["value_b"], np.float32)
    weights_w = np.asarray(inputs["weights_w"], np.float32)
    weights_b = np.asarray(inputs["weights_b"], np.float32)
    offset_w = np.asarray(inputs["offset_w"], np.float32)
    offset_b = np.asarray(inputs["offset_b"], np.float32)
    out_w = np.asarray(inputs["out_w"], np.float32)
    out_b = np.asarray(inputs["out_b"], np.float32)

    w90 = np.concatenate([weights_w, offset_w], 0)            # [90, C]
    b90 = np.concatenate([weights_b, offset_b], 0)            # [90]
    shared = {
        "vwTb": np.ascontiguousarray(value_w.T).astype(ml_dtypes.bfloat16),
        "w90T": np.ascontiguousarray(w90.T),
        "owTb": np.ascontiguousarray(out_w.T).astype(ml_dtypes.bfloat16),
        "b90r": np.broadcast_to(b90, (128, 90)).copy(),
        "bvr": np.broadcast_to(value_b, (128, C)).copy(),
        "outb": out_b.reshape(128, 1).copy(),
        "ident": np.eye(128, dtype=ml_dtypes.bfloat16),
    }
    oneh = np.zeros((128, 8, 128), np.float32)
    for qh in range(8):
        for m in range(128):
            oneh[16 * qh + (m % 16), qh, m] = 1.0
    shared["oneh"] = oneh.reshape(128, 8 * 128)

    in_maps = []
    for core in range(NCORES):
        b_i, sl = core // 4, core % 4
        off = sl * QS
        fr = np.roll(feats[b_i].reshape(C, HW), -off, axis=1)
        fx = np.concatenate([fr, fr[:, : FVLEN - HW]], axis=1)
        an = anchor[b_i, off : off + QS].reshape(NT, 128, 2).transpose(1, 0, 2).reshape(128, NT * 2)
        m = dict(shared)
        m["fvalb"] = np.ascontiguousarray(fx).astype(ml_dtypes.bfloat16)
        m["fproj"] = np.ascontiguousarray(fr[:, :QS])
        m["anch"] = np.ascontiguousarray(an)
        m["rotoff"] = np.full((128, 1), float(off), np.float32)
        in_maps.append(m)
    return in_maps


def kernel(**inputs) -> np.ndarray:
    from concourse.bass_utils import run_bass_kernel_spmd

    if "nc" not in _CACHE:
        _CACHE["nc"] = _build_nc()
    nc = _CACHE["nc"]
    in_maps = _host_prep(inputs)
    res = run_bass_kernel_spmd(nc, in_maps, core_ids=list(range(NCORES)))
    out = np.zeros((B, C, HW), np.float32)
    for core in range(NCORES):
        b_i, sl = core // 4, core % 4
        out[b_i, :, sl * QS : (sl + 1) * QS] = res.results[core]["out"]
    return out.reshape(B, C, H, W)
